# revision 1
# baseline (speedup 1.0000x reference)
"""Trainium2 Bass kernel for the fused 3-modality attention + FFN + softmax model.

Layout strategy: pure data parallel over 8 NeuronCores (batch sharded), all
activations kept FEATURE-MAJOR on chip ([1024 feats = 8 chunks x 128
partitions, tokens in the free dim]) so no on-device transposes are needed.
All GEMMs run in bf16 (1 cycle/row on the PE) accumulating fp32 in PSUM.
LayerNorm reductions over features are ones-vector matmuls on the PE;
per-token scalars are broadcast back across partitions with K=1 expand
matmuls. Host-side prep: transpose activations to [DIM, B] bf16, pre-scale
Wg by 1/3 (modality mean) and Wq/bq by 1/sqrt(HD) (attention scale).
"""

import numpy as np
import ml_dtypes

import concourse.bacc as bacc
import concourse.bass as bass
import concourse.mybir as mybir
import concourse.tile as tile

B, DIM, H, FFN, HD = 16384, 1024, 16, 4096, 64
NCORES = 8
TPC = B // NCORES          # tokens per core
TB = 512                   # token block (matmul moving dim)
KC = DIM // 128            # 8 feature chunks
MC1 = FFN // 128           # 32 ffn chunks
EPS = 1e-5

BF16 = mybir.dt.bfloat16
F32 = mybir.dt.float32
F32R = mybir.dt.float32r
AF = mybir.ActivationFunctionType


def _ln_apply(nc, pp, wk, src_f32, out_bf, g, be, C, cbufs=2):
    """LayerNorm over features (partition x chunk axis); src modified in place.

    src_f32: [128, KC*TB] fp32 tile, out_bf: [128, KC*TB] bf16 tile.
    Per-token scalars live as rows of one packed [8, TB] fp32 tile.
    """
    v, s, te = nc.vector, nc.scalar, nc.tensor
    # bf16 copy of src for the (cheap, 1 cyc/row) column-sum matmuls
    xbc = wk.tile([128, KC * TB], BF16, tag="a4", bufs=cbufs, name="xbc")
    s.activation(xbc[:], src_f32[:], AF.Copy)
    pr1 = pp.tile([16, TB], F32, tag="red", bufs=3, name="pr1")
    for kc in range(KC):
        te.matmul(pr1[0:1, :], C["onecb"][:],
                  xbc[:, kc * TB:(kc + 1) * TB],
                  start=(kc == 0), stop=(kc == KC - 1))
    sq = wk.tile([128, KC * TB], BF16, tag="qb", bufs=1, name="sq")
    s.activation(sq[:], src_f32[:], AF.Square)
    pr2 = pp.tile([16, TB], F32, tag="red", bufs=3, name="pr2")
    for kc in range(KC):
        te.matmul(pr2[0:1, :], C["onecb"][:], sq[:, kc * TB:(kc + 1) * TB],
                  start=(kc == 0), stop=(kc == KC - 1))
    # per-token scalars: separate base-0 tiles (partition-alignment rules)
    mub = wk.tile([1, TB], BF16, tag="ln_mub", bufs=1, name="mub")[:]
    ex2 = wk.tile([1, TB], F32, tag="ln_ex2", bufs=1, name="ex2")[:]
    mu2 = wk.tile([1, TB], F32, tag="ln_mu2", bufs=1, name="mu2")[:]
    var = wk.tile([1, TB], F32, tag="ln_var", bufs=1, name="var")[:]
    sd = wk.tile([1, TB], F32, tag="ln_sd", bufs=1, name="sd")[:]
    rs = wk.tile([1, TB], F32, tag="ln_rs", bufs=1, name="rs")[:]
    rsb = wk.tile([1, TB], BF16, tag="ln_rsb", bufs=1, name="rsb")[:]
    s.activation(mub, pr1[0:1, :], AF.Copy, scale=1.0 / DIM)
    s.activation(ex2, pr2[0:1, :], AF.Copy, scale=1.0 / DIM)
    s.activation(mu2, mub, AF.Square)
    v.tensor_sub(var, ex2, mu2)
    s.activation(sd, var, AF.Sqrt, bias=C["epsc"][:])
    v.reciprocal(rs, sd)
    s.activation(rsb, rs, AF.Copy)
    pmu = pp.tile([128, TB], F32, tag="acc", bufs=5, name="pmu")
    te.matmul(pmu[:], C["onerb"][:], mub, start=True, stop=True)
    prs = pp.tile([128, TB], F32, tag="acc", bufs=5, name="prs")
    te.matmul(prs[:], C["onerb"][:], rsb, start=True, stop=True)
    for kc in range(KC):
        sl = src_f32[:, kc * TB:(kc + 1) * TB]
        v.tensor_sub(sl, sl, pmu[:])
        v.tensor_mul(sl, sl, prs[:])
        s.activation(out_bf[:, kc * TB:(kc + 1) * TB], sl, AF.Identity,
                     scale=g[:, kc:kc + 1], bias=be[:, kc:kc + 1])


def _emit(nc, tc, io, tpc):
    nblk = tpc // TB
    v, s, te = nc.vector, nc.scalar, nc.tensor

    with (
        tc.tile_pool(name="consts", bufs=1) as cp,
        tc.tile_pool(name="psum", bufs=1, space="PSUM") as pp,
    ):
        # ---- constants / small params ----
        C = {}
        for name, shape, dtype in (
            ("Ssel", [128, 128], BF16), ("Eexp", [16, 1024], BF16),
            ("onecb", [128, 1], BF16), ("onerb", [1, 128], BF16),
            ("bg", [128, KC], F32), ("bq", [128, KC], F32),
            ("bk", [128, KC], F32), ("bv", [128, KC], F32),
            ("b1", [128, MC1], F32), ("b2", [128, KC], F32),
            ("g1", [128, KC], F32), ("be1", [128, KC], F32),
            ("g2", [128, KC], F32), ("be2", [128, KC], F32),
            ("Wwt", [128, 3 * KC], BF16), ("bwc", [1, 3], F32),
            ("epsc", [1, 1], F32),
        ):
            t = cp.tile(shape, dtype, name=f"c_{name}")
            nc.sync.dma_start(out=t[:], in_=io[name])
            C[name] = t

        # ---------------- phase A: attention + LN1 ----------------
        with (
            tc.tile_pool(name="wA", bufs=1) as wa,
            tc.tile_pool(name="workA", bufs=1) as wk,
        ):
            wmap = {}
            for wn in ("Wg", "Wq", "Wk", "Wv"):
                wt = wa.tile([128, KC * DIM], BF16, name=f"w_{wn}")
                nc.sync.dma_start(
                    out=wt[:].rearrange("p (c n) -> p c n", n=DIM),
                    in_=io[wn].rearrange("(c p) n -> p c n", p=128))
                wmap[wn] = wt

            def wsl(wn, kc, mc):
                return wmap[wn][:, kc * DIM + mc * 128:kc * DIM + mc * 128 + 128]

            for blk in range(nblk):
                t0 = blk * TB
                ins = {}
                for name in ("m0", "m1", "m2", "dom"):
                    t = wk.tile([128, KC * TB], BF16, tag=f"in_{name}",
                                bufs=(2 if name == "dom" else 1),
                                name=f"{name}_sb")
                    nc.sync.dma_start(
                        out=t[:].rearrange("p (c t) -> p c t", t=TB),
                        in_=io[name].rearrange("(c p) t -> p c t",
                                               p=128)[:, :, t0:t0 + TB])
                    ins[name] = t
                mj = [ins["m0"], ins["m1"], ins["m2"]]
                dom = ins["dom"]

                avg = wk.tile([128, KC * TB], BF16, tag="a4", bufs=2, name="avg")
                v.tensor_add(avg[:], mj[0][:], mj[1][:])
                v.tensor_add(avg[:], avg[:], mj[2][:])

                # global_rep -> qin: (psum + bg) + dom fused on the DVE
                qin = wk.tile([128, KC * TB], BF16, tag="a4", bufs=2, name="qin")
                for mc in range(KC):
                    pg = pp.tile([128, TB], F32, tag="acc", bufs=5, name="pg")
                    for kc in range(KC):
                        te.matmul(pg[:], wsl("Wg", kc, mc),
                                  avg[:, kc * TB:(kc + 1) * TB],
                                  start=(kc == 0), stop=(kc == KC - 1))
                    v.scalar_tensor_tensor(
                        qin[:, mc * TB:(mc + 1) * TB], pg[:],
                        C["bg"][:, mc:mc + 1], dom[:, mc * TB:(mc + 1) * TB],
                        mybir.AluOpType.add, mybir.AluOpType.add)

                qb = wk.tile([128, KC * TB], BF16, tag="qb", bufs=1, name="qb")
                for mc in range(KC):
                    pq = pp.tile([128, TB], F32, tag="acc", bufs=5, name="pq")
                    for kc in range(KC):
                        te.matmul(pq[:], wsl("Wq", kc, mc),
                                  qin[:, kc * TB:(kc + 1) * TB],
                                  start=(kc == 0), stop=(kc == KC - 1))
                    s.activation(qb[:, mc * TB:(mc + 1) * TB], pq[:],
                                 AF.Identity, bias=C["bq"][:, mc:mc + 1])

                # scores[h,t] per modality (Wq/bq pre-scaled by 1/sqrt(HD))
                sc = wk.tile([16, 3 * TB], F32, tag="sc", bufs=1,
                             name="sc")
                for mc in range(KC):
                    pks = []
                    for j in range(3):
                        pks.append(pp.tile([128, TB], F32, tag="acc", bufs=5,
                                           name=f"pk{j}"))
                    for kc in range(KC):
                        for j in range(3):
                            te.matmul(pks[j][:], wsl("Wk", kc, mc),
                                      mj[j][:, kc * TB:(kc + 1) * TB],
                                      start=(kc == 0), stop=(kc == KC - 1))
                    for j in range(3):
                        tm = wk.tile([128, TB], BF16, tag="tm", bufs=2,
                                     name="tm")
                        v.scalar_tensor_tensor(
                            tm[:], pks[j][:], C["bk"][:, mc:mc + 1],
                            qb[:, mc * TB:(mc + 1) * TB],
                            mybir.AluOpType.add, mybir.AluOpType.mult)
                        ps = pp.tile([16, TB], F32, tag="red", bufs=3,
                                     name="ps")
                        te.matmul(ps[:], C["Ssel"][:, mc * 16:(mc + 1) * 16],
                                  tm[:], start=True, stop=True)
                        scj = sc[:, j * TB:(j + 1) * TB]
                        if mc == 0:
                            v.tensor_copy(scj, ps[:])
                        else:
                            v.tensor_add(scj, scj, ps[:])

                # softmax over the 3 modalities (all tiles at base 0)
                mx = wk.tile([16, TB], F32, tag="mx", bufs=1, name="mx")[:]
                sm = wk.tile([16, TB], F32, tag="sm", bufs=1, name="sm")[:]
                rc = wk.tile([16, TB], F32, tag="rc", bufs=1, name="rc")[:]
                v.tensor_max(mx, sc[:, 0:TB], sc[:, TB:2 * TB])
                v.tensor_max(mx, mx, sc[:, 2 * TB:3 * TB])
                for j in range(3):
                    scj = sc[:, j * TB:(j + 1) * TB]
                    v.tensor_sub(scj, scj, mx)
                ab = wk.tile([16, 3 * TB], BF16, tag="ab", bufs=1, name="ab")
                s.activation(ab[:], sc[:], AF.Exp)
                v.tensor_add(sm, ab[:, 0:TB], ab[:, TB:2 * TB])
                v.tensor_add(sm, sm, ab[:, 2 * TB:3 * TB])
                v.reciprocal(rc, sm)
                for j in range(3):
                    abj = ab[:, j * TB:(j + 1) * TB]
                    v.tensor_mul(abj, abj, rc)

                # attnout = sum_j bcast(attn_j) * (m_j @ Wv); bv folds to
                # +bv since sum_j attn_j = 1. All 24 v-matmuls for a chunk
                # are emitted before the attn-dependent expands so the PE
                # never stalls waiting for the softmax.
                xp = wk.tile([128, KC * TB], F32, tag="xp", bufs=1, name="xp")
                for mc in range(KC):
                    pvs = []
                    for j in range(3):
                        pvs.append(pp.tile([128, TB], F32, tag="acc", bufs=5,
                                           name=f"pv{j}"))
                    for kc in range(KC):
                        for j in range(3):
                            te.matmul(pvs[j][:], wsl("Wv", kc, mc),
                                      mj[j][:, kc * TB:(kc + 1) * TB],
                                      start=(kc == 0), stop=(kc == KC - 1))
                    acc = wk.tile([128, TB], F32, tag="acc_s", bufs=2,
                                  name="acc")
                    for j in range(3):
                        vt = wk.tile([128, TB], BF16, tag="vt", bufs=2,
                                     name="vt")
                        s.activation(vt[:], pvs[j][:], AF.Copy)
                        pa = pp.tile([128, TB], F32, tag="red", bufs=3,
                                     name="pa")
                        te.matmul(pa[:], C["Eexp"][:, mc * 128:(mc + 1) * 128],
                                  ab[:, j * TB:(j + 1) * TB],
                                  start=True, stop=True)
                        if j == 0:
                            v.tensor_mul(acc[:], pa[:], vt[:])
                        else:
                            t2 = wk.tile([128, TB], F32, tag="t2", bufs=2,
                                         name="t2")
                            v.tensor_mul(t2[:], pa[:], vt[:])
                            v.tensor_add(acc[:], acc[:], t2[:])
                    v.scalar_tensor_tensor(
                        xp[:, mc * TB:(mc + 1) * TB], acc[:],
                        C["bv"][:, mc:mc + 1], dom[:, mc * TB:(mc + 1) * TB],
                        mybir.AluOpType.add, mybir.AluOpType.add)

                x_bf = wk.tile([128, KC * TB], BF16, tag="xbf", bufs=2,
                               name="x_bf")
                _ln_apply(nc, pp, wk, xp, x_bf, C["g1"], C["be1"], C)
                nc.sync.dma_start(
                    out=io["xs"].rearrange("(c p) t -> p c t",
                                           p=128)[:, :, t0:t0 + TB],
                    in_=x_bf[:].rearrange("p (c t) -> p c t", t=TB))

        # ---------------- phase B: FFN + LN2 + logits ----------------
        with (
            tc.tile_pool(name="wB", bufs=1) as wb,
            tc.tile_pool(name="workB", bufs=1) as wk,
        ):
            w1k = []
            for kc in range(KC):
                t = wb.tile([128, FFN], BF16, name=f"w_W1_{kc}")
                nc.sync.dma_start(
                    out=t[:],
                    in_=io["W1"].rearrange("(c p) n -> p c n",
                                           p=128)[:, kc, :])
                w1k.append(t)

            for blk in range(nblk):
                t0 = blk * TB
                xb = wk.tile([128, KC * TB], BF16, tag="xb", bufs=1, name="xb")
                nc.sync.dma_start(
                    out=xb[:].rearrange("p (c t) -> p c t", t=TB),
                    in_=io["xs"].rearrange("(c p) t -> p c t",
                                           p=128)[:, :, t0:t0 + TB])
                hb = wk.tile([128, MC1 * TB], BF16, tag="hb", bufs=1, name="hb")
                for mc in range(MC1):
                    ph = pp.tile([128, TB], F32, tag="acc", bufs=5, name="ph")
                    for kc in range(KC):
                        te.matmul(ph[:],
                                  w1k[kc][:, mc * 128:mc * 128 + 128],
                                  xb[:, kc * TB:(kc + 1) * TB],
                                  start=(kc == 0), stop=(kc == KC - 1))
                    s.activation(hb[:, mc * TB:(mc + 1) * TB], ph[:], AF.Relu,
                                 bias=C["b1"][:, mc:mc + 1])

                x2 = wk.tile([128, KC * TB], F32, tag="x2", bufs=1, name="x2")
                for mc in range(KC):
                    w2t = wk.tile([128, MC1 * 128], BF16, tag="w2t", bufs=2,
                                  name="w2t")
                    nc.sync.dma_start(
                        out=w2t[:].rearrange("p (c n) -> p c n", n=128),
                        in_=io["W2"].rearrange("(c p) n -> p c n",
                                               p=128)[:, :,
                                                      mc * 128:(mc + 1) * 128])
                    pf = pp.tile([128, TB], F32, tag="acc", bufs=5, name="pf")
                    for kc in range(MC1):
                        te.matmul(pf[:], w2t[:, kc * 128:(kc + 1) * 128],
                                  hb[:, kc * TB:(kc + 1) * TB],
                                  start=(kc == 0), stop=(kc == MC1 - 1))
                    v.scalar_tensor_tensor(
                        x2[:, mc * TB:(mc + 1) * TB], pf[:],
                        C["b2"][:, mc:mc + 1], xb[:, mc * TB:(mc + 1) * TB],
                        mybir.AluOpType.add, mybir.AluOpType.add)

                yb = wk.tile([128, KC * TB], BF16, tag="yb", bufs=1, name="yb")
                _ln_apply(nc, pp, wk, x2, yb, C["g2"], C["be2"], C, cbufs=1)

                # logits: one single-row matmul accumulation per class so
                # every scalar row lives at partition base 0.
                zc, ec = [], []
                for c in range(3):
                    pzc = pp.tile([1, TB], F32, tag="red", bufs=3,
                                  name=f"pz{c}")
                    for kc in range(KC):
                        te.matmul(pzc[:],
                                  C["Wwt"][:, kc * 3 + c:kc * 3 + c + 1],
                                  yb[:, kc * TB:(kc + 1) * TB],
                                  start=(kc == 0), stop=(kc == KC - 1))
                    zt = wk.tile([1, TB], F32, tag=f"z{c}", bufs=1,
                                 name=f"z{c}")
                    s.activation(zt[:], pzc[:], AF.Identity,
                                 bias=C["bwc"][:, c:c + 1])
                    zc.append(zt[:])
                mx3 = wk.tile([1, TB], F32, tag="mx3", bufs=1, name="mx3")[:]
                ss = wk.tile([1, TB], F32, tag="ss", bufs=1, name="ss")[:]
                rr = wk.tile([1, TB], F32, tag="rr", bufs=1, name="rr")[:]
                v.tensor_max(mx3, zc[0], zc[1])
                v.tensor_max(mx3, mx3, zc[2])
                for c in range(3):
                    et = wk.tile([1, TB], F32, tag=f"e{c}", bufs=1,
                                 name=f"e{c}")
                    v.tensor_sub(et[:], zc[c], mx3)
                    s.activation(et[:], et[:], AF.Exp)
                    ec.append(et[:])
                v.tensor_add(ss, ec[0], ec[1])
                v.tensor_add(ss, ss, ec[2])
                v.reciprocal(rr, ss)
                for c in range(3):
                    pt = wk.tile([1, TB], F32, tag=f"p{c}", bufs=1,
                                 name=f"p{c}")
                    v.tensor_mul(pt[:], ec[c], rr)
                    nc.sync.dma_start(
                        out=io["out"][t0:t0 + TB, c:c + 1].rearrange(
                            "t c -> c t"),
                        in_=pt[:])


def build_program(tpc=TPC):
    nc = bacc.Bacc("TRN2", target_bir_lowering=False, debug=False)
    io = {}

    def din(name, shape, dtype):
        io[name] = nc.dram_tensor(name, shape, dtype, kind="ExternalInput").ap()

    for name in ("m0", "m1", "m2", "dom"):
        din(name, [DIM, tpc], BF16)
    for name in ("Wg", "Wq", "Wk", "Wv"):
        din(name, [DIM, DIM], BF16)
    din("W1", [DIM, FFN], BF16)
    din("W2", [FFN, DIM], BF16)
    din("Ssel", [128, 128], BF16)
    din("Eexp", [16, 1024], BF16)
    din("onecb", [128, 1], BF16)
    din("onerb", [1, 128], BF16)
    for name, w in (("bg", KC), ("bq", KC), ("bk", KC), ("bv", KC),
                    ("b1", MC1), ("b2", KC), ("g1", KC), ("be1", KC),
                    ("g2", KC), ("be2", KC)):
        din(name, [128, w], F32)
    din("Wwt", [128, 3 * KC], BF16)
    din("bwc", [1, 3], F32)
    din("epsc", [1, 1], F32)
    io["xs"] = nc.dram_tensor("xs", [DIM, tpc], BF16).ap()
    io["out"] = nc.dram_tensor("out", [tpc, 3], F32,
                               kind="ExternalOutput").ap()

    with tile.TileContext(nc) as tc:
        _emit(nc, tc, io, tpc)
    nc.compile()
    return nc


def _chunk_cols(vec, width):
    """[width*128] host vector -> [128, width] chunk-column layout."""
    return np.ascontiguousarray(vec.reshape(width, 128).T).astype(np.float32)


def prep_host_inputs(inputs, tpc=TPC, ncores=NCORES):
    """Build per-core input maps (host-side shard + transpose + bf16 cast)."""
    bf = ml_dtypes.bfloat16
    f32 = np.float32

    def fm(x):  # [B, DIM] -> [DIM, B] bf16 feature-major
        return np.ascontiguousarray(np.asarray(x, f32).T.astype(bf))

    m0 = fm(inputs["m0"]); m1 = fm(inputs["m1"]); m2 = fm(inputs["m2"])
    dom = fm(inputs["domain_rep"])

    # head-selector S[p, c*16+h] and expander E[h, c*128+p]
    head_of = np.arange(DIM) // HD
    S = np.zeros((128, 128), f32)
    E = np.zeros((16, 1024), f32)
    for c in range(KC):
        for p in range(128):
            h = head_of[c * 128 + p]
            S[p, c * 16 + h] = 1.0
            E[h, c * 128 + p] = 1.0

    consts = {
        "Wg": (np.asarray(inputs["Wg"], f32) / 3.0).astype(bf),
        "Wq": (np.asarray(inputs["Wq"], f32) / np.sqrt(HD)).astype(bf),
        "Wk": np.asarray(inputs["Wk"], f32).astype(bf),
        "Wv": np.asarray(inputs["Wv"], f32).astype(bf),
        "W1": np.asarray(inputs["W1"], f32).astype(bf),
        "W2": np.asarray(inputs["W2"], f32).astype(bf),
        "Ssel": S.astype(bf),
        "Eexp": E.astype(bf),
        "onecb": np.ones((128, 1), f32).astype(bf),
        "onerb": np.ones((1, 128), f32).astype(bf),
        "bg": _chunk_cols(np.asarray(inputs["bg"], f32), KC),
        "bq": _chunk_cols(np.asarray(inputs["bq"], f32) / np.sqrt(HD), KC),
        "bk": _chunk_cols(np.asarray(inputs["bk"], f32), KC),
        "bv": _chunk_cols(np.asarray(inputs["bv"], f32), KC),
        "b1": _chunk_cols(np.asarray(inputs["b1"], f32), MC1),
        "b2": _chunk_cols(np.asarray(inputs["b2"], f32), KC),
        "g1": _chunk_cols(np.asarray(inputs["g1"], f32), KC),
        "be1": _chunk_cols(np.asarray(inputs["beta1"], f32), KC),
        "g2": _chunk_cols(np.asarray(inputs["g2"], f32), KC),
        "be2": _chunk_cols(np.asarray(inputs["beta2"], f32), KC),
        "Wwt": np.ascontiguousarray(
            np.asarray(inputs["Ww"], f32).reshape(KC, 128, 3)
            .transpose(1, 0, 2).reshape(128, 3 * KC)).astype(bf),
        "bwc": np.asarray(inputs["bw"], f32).reshape(1, 3),
        "epsc": np.full((1, 1), EPS, f32),
    }

    in_maps = []
    for c in range(ncores):
        sl = slice(c * tpc, (c + 1) * tpc)
        m = dict(consts)
        m["m0"] = np.ascontiguousarray(m0[:, sl])
        m["m1"] = np.ascontiguousarray(m1[:, sl])
        m["m2"] = np.ascontiguousarray(m2[:, sl])
        m["dom"] = np.ascontiguousarray(dom[:, sl])
        in_maps.append(m)
    return in_maps


def kernel(**inputs):
    from concourse.bass_utils import run_bass_kernel_spmd
    nc = build_program()
    in_maps = prep_host_inputs(inputs)
    res = run_bass_kernel_spmd(nc, in_maps, list(range(NCORES)))
    out = np.concatenate([res.results[c]["out"] for c in range(NCORES)],
                         axis=0)
    return np.ascontiguousarray(out.astype(np.float32))



# revision 24
# speedup vs baseline: 1.1641x; 1.1641x over previous
"""Trainium2 Bass kernel for the fused 3-modality attention + FFN + softmax model.

v2: fp8e4 DoubleRow matmuls (2 k-tiles per PE pass) for all six big GEMMs,
single fused phase (all weights resident in SBUF, no DRAM roundtrip for x),
host-folded Wgq = Wg@Wq/(3*sqrt(HD)) to break the global_rep dependency,
activations carried in "x16 units" (LayerNorm is scale-invariant) so fp8
quantization scales fold into existing eviction scales, and rsqrt/reciprocal
computed as exp(-a*ln(x)) so the ACT engine stays on one table set
(natural_log_exp_and_others) with no table-swap stalls.

Layout: pure data parallel over 8 NeuronCores (batch sharded). Activations
feature-major on chip: [128 partitions, chunk*tokens] with DIM=1024 split
into KC=8 chunks of 128 partitions. LayerNorm reductions over features are
ones-vector matmuls on the PE; per-token scalars broadcast back with K=1
expand matmuls. Final 3-way softmax is done on the host (logits are DMA'd).
"""

import numpy as np
import ml_dtypes

import concourse.bacc as bacc
import concourse.bass as bass
import concourse.mybir as mybir
import concourse.tile as tile

B, DIM, H, FFN, HD = 16384, 1024, 16, 4096, 64
NCORES = 8
TPC = B // NCORES          # tokens per core
TB = 512                   # token block (matmul moving dim)
KC = DIM // 128            # 8 feature chunks
MC1 = FFN // 128           # 32 ffn chunks
EPS = 1e-5

# quantization scales (powers of two)
SA = 16.0                  # activation fp8 scale (x16 units)
SW = 256.0                 # weight fp8 scale for Wk/Wv/W1/W2
SWQ = 2048.0               # weight fp8 scale for Wq' and Wgq
SQK = 4096.0               # folded k-dequant carried inside qb

BF16 = mybir.dt.bfloat16
F32 = mybir.dt.float32
FP8 = mybir.dt.float8e4
AF = mybir.ActivationFunctionType
ALU = mybir.AluOpType
DR = mybir.MatmulPerfMode.DoubleRow


def _c3(t, nper):
    """[128, nchunk*nper] tile AP -> [128, nchunk, nper] view."""
    return t[:].rearrange("p (c n) -> p c n", n=nper)


def _ln_stats(nc, pp, wk, C, src_bf, tag):
    """LayerNorm stats over the feature axis of src_bf [128, KC*TB] (x16
    units). Returns (pmub, prsb): [128, TB] bf16 broadcast tiles of the mean
    (x16) and 1/std (1/16 units) so (src - pmub) * prsb is the true-scale
    normalized value."""
    v, s, te = nc.vector, nc.scalar, nc.tensor
    pr1 = pp.tile([1, TB], F32, tag="red", bufs=3, name=f"pr1{tag}")
    for kc in range(KC):
        te.matmul(pr1[:], C["onecb"][:], src_bf[:, kc * TB:(kc + 1) * TB],
                  start=(kc == 0), stop=(kc == KC - 1))
    pr2 = pp.tile([1, TB], F32, tag="red", bufs=3, name=f"pr2{tag}")
    for p2 in range(KC // 2):
        sq = wk.tile([128, 2 * TB], BF16, tag="sq", bufs=2, name="sq")
        s.activation(sq[:], src_bf[:, 2 * p2 * TB:(2 * p2 + 2) * TB],
                     AF.Square)
        for i in range(2):
            kc = 2 * p2 + i
            te.matmul(pr2[:], C["onecb"][:], sq[:, i * TB:(i + 1) * TB],
                      start=(kc == 0), stop=(kc == KC - 1))
    mub = wk.tile([1, TB], BF16, tag="ln_mub", bufs=1, name="mub")[:]
    ex2 = wk.tile([1, TB], F32, tag="ln_ex2", bufs=1, name="ex2")[:]
    mu2 = wk.tile([1, TB], F32, tag="ln_mu2", bufs=1, name="mu2")[:]
    var = wk.tile([1, TB], F32, tag="ln_var", bufs=1, name="var")[:]
    lnv = wk.tile([1, TB], F32, tag="ln_lnv", bufs=1, name="lnv")[:]
    rsb = wk.tile([1, TB], BF16, tag="ln_rsb", bufs=1, name="rsb")[:]
    s.activation(mub, pr1[:], AF.Copy, scale=1.0 / DIM)
    s.activation(ex2, pr2[:], AF.Copy, scale=1.0 / DIM)
    s.activation(mu2, mub, AF.Square)
    v.tensor_sub(var, ex2, mu2)
    # rs = exp(-0.5*ln(var + 256*eps)) : stays on the ln/exp ACT table set
    s.activation(lnv, var, AF.Ln, bias=C["epsc"][:])
    s.activation(rsb, lnv, AF.Exp, scale=-0.5)
    pmu = pp.tile([128, TB], F32, tag="red", bufs=3, name=f"pmu{tag}")
    te.matmul(pmu[:], C["onerb"][:], mub, start=True, stop=True)
    prs = pp.tile([128, TB], F32, tag="red", bufs=3, name=f"prs{tag}")
    te.matmul(prs[:], C["onerb"][:], rsb, start=True, stop=True)
    pmub = wk.tile([128, TB], BF16, tag="pmub", bufs=2, name="pmub")
    s.activation(pmub[:], pmu[:], AF.Copy)
    prsb = wk.tile([128, TB], BF16, tag="prsb", bufs=2, name="prsb")
    s.activation(prsb[:], prs[:], AF.Copy)
    return pmub, prsb


def _emit(nc, tc, io, tpc):
    nblk = tpc // TB
    v, s, te = nc.vector, nc.scalar, nc.tensor

    with (
        tc.tile_pool(name="consts", bufs=1) as cp,
        tc.tile_pool(name="weights", bufs=1) as wp,
        tc.tile_pool(name="psum", bufs=1, space="PSUM") as pp,
        tc.tile_pool(name="work", bufs=1) as wk,
    ):
        # ---- small constants ----
        C = {}
        for name, shape, dtype in (
            ("Ssel", [128, 128], BF16), ("Eexp", [16, 1024], BF16),
            ("onecb", [128, 1], BF16), ("onerb", [1, 128], BF16),
            ("qbias", [128, KC], F32), ("bk", [128, KC], F32),
            ("bv", [128, KC], F32),
            ("b1", [128, MC1], F32), ("b2", [128, KC], F32),
            ("g1", [128, KC], F32), ("be1", [128, KC], F32),
            ("g2", [128, KC], F32), ("be2", [128, KC], F32),
            ("Wwt", [128, 3 * KC], BF16), ("bw", [3, 1], F32),
            ("epsc", [1, 1], F32),
        ):
            t = cp.tile(shape, dtype, name=f"c_{name}")
            nc.sync.dma_start(out=t[:], in_=io[name])
            C[name] = t

        # ---- resident fp8 weights (chunk-major: w[p, kc*N + n]) ----
        W = {}
        for name, width in (("Wqg", 2 * KC * DIM), ("Wk", KC * DIM),
                            ("Wv", KC * DIM)):
            t = wp.tile([128, width], FP8, name=f"w_{name}")
            nc.sync.dma_start(out=t[:], in_=io[name])
            W[name] = t
        wqg = _c3(W["Wqg"], DIM)
        wkc = _c3(W["Wk"], DIM)
        wvc = _c3(W["Wv"], DIM)

        for blk in range(nblk):
            t0 = blk * KC * TB

            # ---- per-block inputs (block-major contiguous in DRAM) ----
            qmv = wk.tile([128, 2 * KC * TB], FP8, tag="qmv", bufs=1,
                          name="qmv")
            nc.sync.dma_start(out=qmv[:],
                              in_=io["qmv"][:, 2 * t0:2 * t0 + 2 * KC * TB])
            mj = []
            for j in range(3):
                t = wk.tile([128, KC * TB], FP8, tag=f"m{j}", bufs=2,
                            name=f"m{j}")
                nc.sync.dma_start(out=t[:],
                                  in_=io[f"m{j}"][:, t0:t0 + KC * TB])
                mj.append(t)
            domb = wk.tile([128, KC * TB], BF16, tag="domb", bufs=1,
                           name="domb")
            nc.sync.dma_start(out=domb[:],
                              in_=io["domb"][:, t0:t0 + KC * TB])
            qmv3 = _c3(qmv, TB)
            mj3 = [_c3(t, TB) for t in mj]

            # ---- q = dom@Wq' + msum@Wgq (one 2048-deep contraction) ----
            qb = wk.tile([128, KC * TB], BF16, tag="qb", bufs=1, name="qb")
            for mc in range(KC):
                pq = pp.tile([128, TB], F32, tag="acc", bufs=5, name="pq")
                for kp in range(KC):
                    te.matmul(pq[:],
                              wqg[:, 2 * kp:2 * kp + 2,
                                  mc * 128:(mc + 1) * 128],
                              qmv3[:, 2 * kp:2 * kp + 2, :],
                              start=(kp == 0), stop=(kp == KC - 1),
                              perf_mode=DR)
                s.activation(qb[:, mc * TB:(mc + 1) * TB], pq[:],
                             AF.Identity, scale=1.0 / (SA * SWQ * SQK),
                             bias=C["qbias"][:, mc:mc + 1])

            # ---- scores: sc_j[h,t] accumulated over chunks on the PE ----
            scs = []
            for j in range(3):
                scs.append(pp.tile([16, TB], F32, tag="red", bufs=3,
                                   name=f"sc{j}"))
            for mc in range(KC):
                pks = []
                for j in range(3):
                    pks.append(pp.tile([128, TB], F32, tag="acc", bufs=5,
                                       name=f"pk{j}"))
                for kp in range(KC // 2):
                    for j in range(3):
                        te.matmul(pks[j][:],
                                  wkc[:, 2 * kp:2 * kp + 2,
                                      mc * 128:(mc + 1) * 128],
                                  mj3[j][:, 2 * kp:2 * kp + 2, :],
                                  start=(kp == 0), stop=(kp == KC // 2 - 1),
                                  perf_mode=DR)
                for j in range(3):
                    tm = wk.tile([128, TB], BF16, tag="tm", bufs=2, name="tm")
                    v.scalar_tensor_tensor(
                        tm[:], pks[j][:], C["bk"][:, mc:mc + 1],
                        qb[:, mc * TB:(mc + 1) * TB], ALU.add, ALU.mult)
                    te.matmul(scs[j][:], C["Ssel"][:, mc * 16:(mc + 1) * 16],
                              tm[:], start=(mc == 0), stop=(mc == KC - 1),
                              skip_group_check=True)

            # ---- softmax over the 3 modalities (no max-sub needed) ----
            ab = wk.tile([16, 3 * TB], BF16, tag="ab", bufs=1, name="ab")
            for j in range(3):
                s.activation(ab[:, j * TB:(j + 1) * TB], scs[j][:], AF.Exp)
            sm = wk.tile([16, TB], F32, tag="sm", bufs=1, name="sm")[:]
            lsm = wk.tile([16, TB], F32, tag="lsm", bufs=1, name="lsm")[:]
            rc = wk.tile([16, TB], BF16, tag="rc", bufs=1, name="rc")[:]
            v.tensor_add(sm, ab[:, 0:TB], ab[:, TB:2 * TB])
            v.tensor_add(sm, sm, ab[:, 2 * TB:3 * TB])
            s.activation(lsm, sm, AF.Ln)
            s.activation(rc, lsm, AF.Exp, scale=-1.0)
            for j in range(3):
                abj = ab[:, j * TB:(j + 1) * TB]
                v.tensor_mul(abj, abj, rc)

            # ---- attention-weighted V + residual -> xp (x16 units) ----
            xp = wk.tile([128, KC * TB], BF16, tag="xp", bufs=1, name="xp")
            for mc in range(KC):
                pvs = []
                for j in range(3):
                    pvs.append(pp.tile([128, TB], F32, tag="acc", bufs=5,
                                       name=f"pv{j}"))
                for kp in range(KC // 2):
                    for j in range(3):
                        te.matmul(pvs[j][:],
                                  wvc[:, 2 * kp:2 * kp + 2,
                                      mc * 128:(mc + 1) * 128],
                                  mj3[j][:, 2 * kp:2 * kp + 2, :],
                                  start=(kp == 0), stop=(kp == KC // 2 - 1),
                                  perf_mode=DR)
                acc = wk.tile([128, TB], BF16, tag="acc_s", bufs=2,
                              name="acc")
                for j in range(3):
                    vt = wk.tile([128, TB], BF16, tag="vt", bufs=2, name="vt")
                    v.tensor_copy(vt[:], pvs[j][:])
                    pa = pp.tile([128, TB], F32, tag="red", bufs=3, name="pa")
                    te.matmul(pa[:], C["Eexp"][:, mc * 128:(mc + 1) * 128],
                              ab[:, j * TB:(j + 1) * TB],
                              start=True, stop=True)
                    if j == 0:
                        v.tensor_mul(acc[:], pa[:], vt[:])
                    else:
                        t2 = wk.tile([128, TB], BF16, tag="t2", bufs=2,
                                     name="t2")
                        v.tensor_mul(t2[:], pa[:], vt[:])
                        v.tensor_add(acc[:], acc[:], t2[:])
                # domb carries 16*(dom + bv); acc is 4096*sum_j a_j v_j
                v.scalar_tensor_tensor(
                    xp[:, mc * TB:(mc + 1) * TB], acc[:],
                    SA / SQK, domb[:, mc * TB:(mc + 1) * TB],
                    ALU.mult, ALU.add)

            # ---- LN1 -> xf = 16*LN1(x) in bf16 (FFN runs bf16 for accuracy)
            pmub, prsb = _ln_stats(nc, pp, wk, C, xp, "a")
            xf = wk.tile([128, KC * TB], BF16, tag="xf", bufs=1, name="xf")
            for kc in range(KC):
                xn = wk.tile([128, TB], BF16, tag="xn", bufs=2, name="xn")
                v.tensor_sub(xn[:], xp[:, kc * TB:(kc + 1) * TB], pmub[:])
                v.tensor_mul(xn[:], xn[:], prsb[:])
                s.activation(xf[:, kc * TB:(kc + 1) * TB], xn[:],
                             AF.Identity, scale=C["g1"][:, kc:kc + 1],
                             bias=C["be1"][:, kc:kc + 1])

            # ---- FFN1 (bf16): h = 16*relu(x@W1 + b1); W1 streamed ----
            hb = wk.tile([128, MC1 * TB], BF16, tag="hb", bufs=1, name="hb")
            for mc in range(MC1):
                w1t = wk.tile([128, KC * 128], BF16, tag="w1s", bufs=4,
                              name="w1t")
                nc.sync.dma_start(
                    out=w1t[:],
                    in_=io["W1"][:, mc * KC * 128:(mc + 1) * KC * 128])
                ph = pp.tile([128, TB], F32, tag="acc", bufs=5, name="ph")
                for kc in range(KC):
                    te.matmul(ph[:], w1t[:, kc * 128:(kc + 1) * 128],
                              xf[:, kc * TB:(kc + 1) * TB],
                              start=(kc == 0), stop=(kc == KC - 1))
                s.activation(hb[:, mc * TB:(mc + 1) * TB], ph[:],
                             AF.Relu, bias=C["b1"][:, mc:mc + 1])

            # ---- FFN2 (bf16, W2 streamed) + residual -> x2 (x16 units) ----
            x2 = wk.tile([128, KC * TB], BF16, tag="x2", bufs=1, name="x2")
            for mp in range(KC // 2):
                tf = wk.tile([128, 2 * TB], BF16, tag="tf", bufs=1, name="tf")
                for half in range(2):
                    mc = 2 * mp + half
                    w2t = wk.tile([128, MC1 * 128], BF16, tag="w2s", bufs=2,
                                  name="w2t")
                    nc.sync.dma_start(
                        out=w2t[:],
                        in_=io["W2"][:, mc * MC1 * 128:(mc + 1) * MC1 * 128])
                    pf = pp.tile([128, TB], F32, tag="acc", bufs=5,
                                 name="pf")
                    for kc in range(MC1):
                        te.matmul(pf[:], w2t[:, kc * 128:(kc + 1) * 128],
                                  hb[:, kc * TB:(kc + 1) * TB],
                                  start=(kc == 0), stop=(kc == MC1 - 1))
                    s.activation(tf[:, half * TB:(half + 1) * TB], pf[:],
                                 AF.Identity, bias=C["b2"][:, mc:mc + 1])
                v.tensor_add(x2[:, 2 * mp * TB:(2 * mp + 2) * TB], tf[:],
                             xf[:, 2 * mp * TB:(2 * mp + 2) * TB])

            # ---- LN2 -> yb, logits accumulated per chunk ----
            pmub2, prsb2 = _ln_stats(nc, pp, wk, C, x2, "b")
            pz = pp.tile([3, TB], F32, tag="red", bufs=3, name="pz")
            for kc in range(KC):
                yn = wk.tile([128, TB], BF16, tag="xn", bufs=2, name="yn")
                v.tensor_sub(yn[:], x2[:, kc * TB:(kc + 1) * TB], pmub2[:])
                v.tensor_mul(yn[:], yn[:], prsb2[:])
                yb = wk.tile([128, TB], BF16, tag="yb", bufs=2, name="yb")
                s.activation(yb[:], yn[:], AF.Identity,
                             scale=C["g2"][:, kc:kc + 1],
                             bias=C["be2"][:, kc:kc + 1])
                te.matmul(pz[:], C["Wwt"][:, kc * 3:(kc + 1) * 3], yb[:],
                          start=(kc == 0), stop=(kc == KC - 1),
                          skip_group_check=True)
            zt = wk.tile([3, TB], F32, tag="zt", bufs=1, name="zt")
            s.activation(zt[:], pz[:], AF.Identity, bias=C["bw"][:])
            nc.sync.dma_start(out=io["zout"][:, blk * TB:(blk + 1) * TB],
                              in_=zt[:])


def build_program(tpc=TPC):
    nc = bacc.Bacc("TRN2", target_bir_lowering=False, debug=False)
    io = {}

    def din(name, shape, dtype):
        io[name] = nc.dram_tensor(name, shape, dtype, kind="ExternalInput").ap()

    nblk = tpc // TB
    din("qmv", [128, nblk * 2 * KC * TB], FP8)
    for j in range(3):
        din(f"m{j}", [128, nblk * KC * TB], FP8)
    din("domb", [128, nblk * KC * TB], BF16)
    din("Wqg", [128, 2 * KC * DIM], FP8)
    din("Wk", [128, KC * DIM], FP8)
    din("Wv", [128, KC * DIM], FP8)
    din("W1", [128, MC1 * KC * 128], BF16)
    din("W2", [128, KC * MC1 * 128], BF16)
    din("Ssel", [128, 128], BF16)
    din("Eexp", [16, 1024], BF16)
    din("onecb", [128, 1], BF16)
    din("onerb", [1, 128], BF16)
    for name, w in (("qbias", KC), ("bk", KC), ("bv", KC), ("b1", MC1),
                    ("b2", KC), ("g1", KC), ("be1", KC), ("g2", KC),
                    ("be2", KC)):
        din(name, [128, w], F32)
    din("Wwt", [128, 3 * KC], BF16)
    din("bw", [3, 1], F32)
    din("epsc", [1, 1], F32)
    io["zout"] = nc.dram_tensor("zout", [3, tpc], F32,
                                kind="ExternalOutput").ap()

    with tile.TileContext(nc) as tc:
        _emit(nc, tc, io, tpc)
    nc.compile()
    return nc


def _chunk_cols(vec, width):
    """[width*128] host vector -> [128, width] chunk-column layout."""
    return np.ascontiguousarray(
        np.asarray(vec, np.float32).reshape(width, 128).T)


def _chunk_major(w, scale):
    """[Din, N] weight -> [128, (Din/128)*N] fp8 chunk-major, scaled."""
    f8 = ml_dtypes.float8_e4m3
    din, n = w.shape
    kc = din // 128
    out = (np.asarray(w, np.float32) * scale).reshape(kc, 128, n)
    out = out.transpose(1, 0, 2).reshape(128, kc * n)
    return np.ascontiguousarray(np.clip(out, -224, 224)).astype(f8)


def _stream_layout(w, nin, nout):
    """[Din, Dout] -> [128, nout*nin*128] bf16 per-out-chunk streaming
    tiles: t[p, mc*nin*128 + kc*128 + n] = w[kc*128+p, mc*128+n]."""
    out = np.asarray(w, np.float32).reshape(nin, 128, nout, 128)
    out = out.transpose(1, 2, 0, 3).reshape(128, nout * nin * 128)
    return np.ascontiguousarray(out).astype(ml_dtypes.bfloat16)


def _act_blocks(x, tpc, scale, dtype):
    """[B, DIM] -> per-core list of [128, nblk*KC*TB] block-major tiles."""
    nblk = tpc // TB
    xs = (np.asarray(x, np.float32).T * scale)        # [DIM, B]
    out = []
    for c in range(xs.shape[1] // tpc):
        xc = xs[:, c * tpc:(c + 1) * tpc]             # [DIM, tpc]
        # a[p, blk*KC*TB + kc*TB + t] = xc[kc*128+p, blk*TB+t]
        a = xc.reshape(KC, 128, nblk, TB).transpose(1, 2, 0, 3)
        a = np.ascontiguousarray(a.reshape(128, nblk * KC * TB))
        if dtype == "f8":
            a = np.clip(a, -224, 224).astype(ml_dtypes.float8_e4m3)
        else:
            a = a.astype(ml_dtypes.bfloat16)
        out.append(a)
    return out


def prep_host_inputs(inputs, tpc=TPC, ncores=NCORES):
    f32 = np.float32
    bf = ml_dtypes.bfloat16
    rt = 1.0 / np.sqrt(HD)

    Wq = np.asarray(inputs["Wq"], f32) * rt
    Wg = np.asarray(inputs["Wg"], f32)
    Wgq = (Wg @ Wq) / 3.0
    qbias = (np.asarray(inputs["bg"], f32) @ Wq
             + np.asarray(inputs["bq"], f32) * rt)

    # head-selector S[p, mc*16+h] and expander E[h, mc*128+p]
    head_of = np.arange(DIM) // HD
    S = np.zeros((128, 128), f32)
    E = np.zeros((16, 1024), f32)
    for c in range(KC):
        for p in range(128):
            h = head_of[c * 128 + p]
            S[p, c * 16 + h] = 1.0
            E[h, c * 128 + p] = 1.0

    consts = {
        "Wqg": np.concatenate([_chunk_major(Wq, SWQ),
                               _chunk_major(Wgq, SWQ)], axis=1),
        "Wk": _chunk_major(np.asarray(inputs["Wk"], f32), SW),
        "Wv": _chunk_major(np.asarray(inputs["Wv"], f32), SW),
        "W1": _stream_layout(np.asarray(inputs["W1"], f32), KC, MC1),
        "W2": _stream_layout(np.asarray(inputs["W2"], f32), MC1, KC),
        "Ssel": S.astype(bf),
        "Eexp": E.astype(bf),
        "onecb": np.ones((128, 1), f32).astype(bf),
        "onerb": np.ones((1, 128), f32).astype(bf),
        "qbias": _chunk_cols(qbias / SQK, KC),
        "bk": _chunk_cols(np.asarray(inputs["bk"], f32) * SQK, KC),
        "bv": _chunk_cols(np.asarray(inputs["bv"], f32) * SA, KC),
        "b1": _chunk_cols(np.asarray(inputs["b1"], f32) * SA, MC1),
        "b2": _chunk_cols(np.asarray(inputs["b2"], f32) * SA, KC),
        "g1": _chunk_cols(np.asarray(inputs["g1"], f32) * SA, KC),
        "be1": _chunk_cols(np.asarray(inputs["beta1"], f32) * SA, KC),
        "g2": _chunk_cols(np.asarray(inputs["g2"], f32), KC),
        "be2": _chunk_cols(np.asarray(inputs["beta2"], f32), KC),
        "Wwt": np.ascontiguousarray(
            np.asarray(inputs["Ww"], f32).reshape(KC, 128, 3)
            .transpose(1, 0, 2).reshape(128, 3 * KC)).astype(bf),
        "bw": np.asarray(inputs["bw"], f32).reshape(3, 1),
        "epsc": np.full((1, 1), 256.0 * EPS, f32),
    }

    m0 = np.asarray(inputs["m0"], f32)
    m1 = np.asarray(inputs["m1"], f32)
    m2 = np.asarray(inputs["m2"], f32)
    dom = np.asarray(inputs["domain_rep"], f32)
    msum = m0 + m1 + m2

    m0b = _act_blocks(m0, tpc, SA, "f8")
    m1b = _act_blocks(m1, tpc, SA, "f8")
    m2b = _act_blocks(m2, tpc, SA, "f8")
    domf = _act_blocks(dom, tpc, SA, "f8")
    msf = _act_blocks(msum, tpc, SA, "f8")
    # domb carries the attention-path residual plus bv (sum_j attn_j == 1)
    domb = _act_blocks(dom + np.asarray(inputs["bv"], f32)[None, :],
                       tpc, SA, "bf")

    nblk = tpc // TB
    in_maps = []
    for c in range(ncores):
        m = dict(consts)
        # qmv: per block, dom's 8 chunks then msum's 8 chunks
        q = np.empty((128, nblk * 2 * KC * TB), ml_dtypes.float8_e4m3)
        for b_ in range(nblk):
            q[:, b_ * 2 * KC * TB:b_ * 2 * KC * TB + KC * TB] = \
                domf[c][:, b_ * KC * TB:(b_ + 1) * KC * TB]
            q[:, b_ * 2 * KC * TB + KC * TB:(b_ + 1) * 2 * KC * TB] = \
                msf[c][:, b_ * KC * TB:(b_ + 1) * KC * TB]
        m["qmv"] = q
        m["m0"] = m0b[c]
        m["m1"] = m1b[c]
        m["m2"] = m2b[c]
        m["domb"] = domb[c]
        in_maps.append(m)
    return in_maps


def kernel(**inputs):
    from concourse.bass_utils import run_bass_kernel_spmd
    nc = build_program()
    in_maps = prep_host_inputs(inputs)
    res = run_bass_kernel_spmd(nc, in_maps, list(range(NCORES)))
    zs = [res.results[c]["zout"] for c in range(NCORES)]
    z = np.concatenate([np.asarray(zc, np.float32).T for zc in zs], axis=0)
    z = z - z.max(axis=1, keepdims=True)
    e = np.exp(z)
    return np.ascontiguousarray(
        (e / e.sum(axis=1, keepdims=True)).astype(np.float32))


# revision 27
# speedup vs baseline: 1.3115x; 1.1267x over previous
"""Trainium2 Bass kernel for the fused 3-modality attention + FFN + softmax model.

v2: fp8e4 DoubleRow matmuls (2 k-tiles per PE pass) for all six big GEMMs,
single fused phase (all weights resident in SBUF, no DRAM roundtrip for x),
host-folded Wgq = Wg@Wq/(3*sqrt(HD)) to break the global_rep dependency,
activations carried in "x16 units" (LayerNorm is scale-invariant) so fp8
quantization scales fold into existing eviction scales, and rsqrt/reciprocal
computed as exp(-a*ln(x)) so the ACT engine stays on one table set
(natural_log_exp_and_others) with no table-swap stalls.

Layout: pure data parallel over 8 NeuronCores (batch sharded). Activations
feature-major on chip: [128 partitions, chunk*tokens] with DIM=1024 split
into KC=8 chunks of 128 partitions. LayerNorm reductions over features are
ones-vector matmuls on the PE; per-token scalars broadcast back with K=1
expand matmuls. Final 3-way softmax is done on the host (logits are DMA'd).
"""

import numpy as np
import ml_dtypes

import concourse.bacc as bacc
import concourse.bass as bass
import concourse.mybir as mybir
import concourse.tile as tile

B, DIM, H, FFN, HD = 16384, 1024, 16, 4096, 64
NCORES = 8
TPC = B // NCORES          # tokens per core
TB = 512                   # token block (matmul moving dim)
KC = DIM // 128            # 8 feature chunks
MC1 = FFN // 128           # 32 ffn chunks
EPS = 1e-5

# quantization scales (powers of two)
SA = 16.0                  # activation fp8 scale (x16 units)
SW = 256.0                 # weight fp8 scale for Wk/Wv/W1/W2
SWQ = 2048.0               # weight fp8 scale for Wq' and Wgq
SQK = 4096.0               # folded k-dequant carried inside qb

BF16 = mybir.dt.bfloat16
F32 = mybir.dt.float32
FP8 = mybir.dt.float8e4
AF = mybir.ActivationFunctionType
ALU = mybir.AluOpType
DR = mybir.MatmulPerfMode.DoubleRow


def _c3(t, nper):
    """[128, nchunk*nper] tile AP -> [128, nchunk, nper] view."""
    return t[:].rearrange("p (c n) -> p c n", n=nper)


def _ln_sums(nc, pp, wk, C, src_bf, tag):
    """Feature-axis sum (rows 0) and sum-of-squares (rows 32) matmuls into
    ONE psum bank via col-tiling. Returns the shared psum tile."""
    s, te = nc.scalar, nc.tensor
    pr = pp.tile([33, TB], F32, tag="ps", bufs=PSB, name=f"pr{tag}")
    for kc in range(KC):
        te.matmul(pr[0:1, :], C["onecb"][:], src_bf[:, kc * TB:(kc + 1) * TB],
                  start=(kc == 0), stop=(kc == KC - 1),
                  skip_group_check=True)
    for p2 in range(KC // 2):
        sq = wk.tile([128, 2 * TB], BF16, tag="sq", bufs=2, name="sq")
        s.activation(sq[:], src_bf[:, 2 * p2 * TB:(2 * p2 + 2) * TB],
                     AF.Square)
        for i in range(2):
            kc = 2 * p2 + i
            te.matmul(pr[32:33, :], C["onecb"][:],
                      sq[:, i * TB:(i + 1) * TB],
                      start=(kc == 0), stop=(kc == KC - 1),
                      skip_group_check=True)
    return pr


def _ln_finish(nc, pp, wk, C, pr, tag):
    """Stats scalar chain + broadcast; returns (pmub, prsb) bf16 [128, TB]."""
    v, s, te = nc.vector, nc.scalar, nc.tensor
    mub = wk.tile([1, TB], BF16, tag="ln_mub", bufs=1, name="mub")[:]
    ex2 = wk.tile([1, TB], F32, tag="ln_ex2", bufs=1, name="ex2")[:]
    mu2 = wk.tile([1, TB], F32, tag="ln_mu2", bufs=1, name="mu2")[:]
    var = wk.tile([1, TB], F32, tag="ln_var", bufs=1, name="var")[:]
    lnv = wk.tile([1, TB], F32, tag="ln_lnv", bufs=1, name="lnv")[:]
    rsb = wk.tile([1, TB], BF16, tag="ln_rsb", bufs=1, name="rsb")[:]
    s.activation(mub, pr[0:1, :], AF.Copy, scale=1.0 / DIM)
    s.activation(ex2, pr[32:33, :], AF.Copy, scale=1.0 / DIM)
    s.activation(mu2, mub, AF.Square)
    v.tensor_sub(var, ex2, mu2)
    # rs = exp(-0.5*ln(var + 256*eps)) : stays on the ln/exp ACT table set
    s.activation(lnv, var, AF.Ln, bias=C["epsc"][:])
    s.activation(rsb, lnv, AF.Exp, scale=-0.5)
    pmu = pp.tile([128, TB], F32, tag="ps", bufs=PSB, name=f"pmu{tag}")
    te.matmul(pmu[:], C["onerb"][:], mub, start=True, stop=True)
    prs = pp.tile([128, TB], F32, tag="ps", bufs=PSB, name=f"prs{tag}")
    te.matmul(prs[:], C["onerb"][:], rsb, start=True, stop=True)
    pmub = wk.tile([128, TB], BF16, tag="pmub", bufs=2, name="pmub")
    s.activation(pmub[:], pmu[:], AF.Copy)
    prsb = wk.tile([128, TB], BF16, tag="prsb", bufs=2, name="prsb")
    s.activation(prsb[:], prs[:], AF.Copy)
    return pmub, prsb


PSB = 8  # single rotating psum tag: all tiles are one bank each


def _emit(nc, tc, io, tpc):
    nblk = tpc // TB
    v, s, te = nc.vector, nc.scalar, nc.tensor

    with (
        tc.tile_pool(name="consts", bufs=1) as cp,
        tc.tile_pool(name="weights", bufs=1) as wp,
        tc.tile_pool(name="psum", bufs=1, space="PSUM") as pp,
        tc.tile_pool(name="work", bufs=1) as wk,
    ):
        # pin the ACT table to natural_log_exp_and_others (covers Copy/
        # Identity/Relu/Square/Exp/Ln) so the greedy per-func chooser never
        # flip-flops tables mid-kernel.
        nc.scalar.add_instruction(mybir.InstLoadActFuncSet(
            name=nc.get_next_instruction_name(), ins=[], outs=[],
            act_func_set_id=6))

        # ---- small constants ----
        C = {}
        for name, shape, dtype in (
            ("Ssel", [128, 128], BF16), ("Eexp", [16, 1024], BF16),
            ("onecb", [128, 1], BF16), ("onerb", [1, 128], BF16),
            ("qbias", [128, KC], F32), ("bk", [128, KC], F32),
            ("b1", [128, MC1], F32), ("b2", [128, KC], F32),
            ("g1", [128, KC], F32), ("be1", [128, KC], F32),
            ("g2", [128, KC], F32), ("be2", [128, KC], F32),
            ("Wwt", [128, 3 * KC], BF16), ("bw", [3, 1], F32),
            ("epsc", [1, 1], F32),
        ):
            t = cp.tile(shape, dtype, name=f"c_{name}")
            nc.sync.dma_start(out=t[:], in_=io[name])
            C[name] = t

        # ---- resident fp8 weights (chunk-major: w[p, kc*N + n]) ----
        W = {}
        for name, width in (("Wqg", 2 * KC * DIM), ("Wk", KC * DIM),
                            ("Wv", KC * DIM)):
            t = wp.tile([128, width], FP8, name=f"w_{name}")
            nc.sync.dma_start(out=t[:], in_=io[name])
            W[name] = t
        wqg = _c3(W["Wqg"], DIM)
        wkc = _c3(W["Wk"], DIM)
        wvc = _c3(W["Wv"], DIM)

        def emit_inputs_q(blk):
            """DMA block inputs; q = dom@Wq' + msum@Wgq (2048-deep)."""
            t0 = blk * KC * TB
            st = {}
            qmv = wk.tile([128, 2 * KC * TB], FP8, tag="qmv", bufs=1,
                          name="qmv")
            nc.sync.dma_start(out=qmv[:],
                              in_=io["qmv"][:, 2 * t0:2 * t0 + 2 * KC * TB])
            mj = []
            for j in range(3):
                t = wk.tile([128, KC * TB], FP8, tag=f"m{j}", bufs=2,
                            name=f"m{j}")
                nc.sync.dma_start(out=t[:],
                                  in_=io[f"m{j}"][:, t0:t0 + KC * TB])
                mj.append(t)
            domb = wk.tile([128, KC * TB], BF16, tag="domb", bufs=1,
                           name="domb")
            nc.sync.dma_start(out=domb[:],
                              in_=io["domb"][:, t0:t0 + KC * TB])
            st["qmv3"] = _c3(qmv, TB)
            st["mj3"] = [_c3(t, TB) for t in mj]
            st["domb"] = domb
            qb = wk.tile([128, KC * TB], BF16, tag="qb", bufs=1, name="qb")
            for mc in range(KC):
                pq = pp.tile([128, TB], F32, tag="ps", bufs=PSB, name="pq")
                for kp in range(KC):
                    te.matmul(pq[:],
                              wqg[:, 2 * kp:2 * kp + 2,
                                  mc * 128:(mc + 1) * 128],
                              st["qmv3"][:, 2 * kp:2 * kp + 2, :],
                              start=(kp == 0), stop=(kp == KC - 1),
                              perf_mode=DR)
                s.activation(qb[:, mc * TB:(mc + 1) * TB], pq[:],
                             AF.Identity, scale=1.0 / (SA * SWQ * SQK),
                             bias=C["qbias"][:, mc:mc + 1])
            st["qb"] = qb
            return st

        def emit_k_softmax(blk, st):
            """Scores into one col-tiled psum bank; selects lag one chunk so
            the PE never waits on the tm STT; then the 3-way softmax."""
            qb = st["qb"]
            sca = pp.tile([96, TB], F32, tag="ps", bufs=PSB, name="sca")
            pend = None
            for mc in range(KC):
                pks = []
                for j in range(3):
                    pks.append(pp.tile([128, TB], F32, tag="ps", bufs=PSB,
                                       name=f"pk{j}"))
                for kp in range(KC // 2):
                    for j in range(3):
                        te.matmul(pks[j][:],
                                  wkc[:, 2 * kp:2 * kp + 2,
                                      mc * 128:(mc + 1) * 128],
                                  st["mj3"][j][:, 2 * kp:2 * kp + 2, :],
                                  start=(kp == 0), stop=(kp == KC // 2 - 1),
                                  perf_mode=DR)
                tms = []
                for j in range(3):
                    tm = wk.tile([128, TB], BF16, tag="tm", bufs=6, name="tm")
                    v.scalar_tensor_tensor(
                        tm[:], pks[j][:], C["bk"][:, mc:mc + 1],
                        qb[:, mc * TB:(mc + 1) * TB], ALU.add, ALU.mult)
                    tms.append(tm)

                def sel(pmc, ptms):
                    for j in range(3):
                        te.matmul(sca[32 * j:32 * j + 16, :],
                                  C["Ssel"][:, pmc * 16:(pmc + 1) * 16],
                                  ptms[j][:],
                                  start=(pmc == 0),
                                  stop=(pmc == KC - 1),
                                  skip_group_check=True)
                if pend is not None:
                    sel(*pend)
                pend = (mc, tms)
            sel(*pend)

            ab = wk.tile([16, 3 * TB], BF16, tag="ab", bufs=1, name="ab")
            for j in range(3):
                s.activation(ab[:, j * TB:(j + 1) * TB],
                             sca[32 * j:32 * j + 16, :], AF.Exp)
            sm = wk.tile([16, TB], BF16, tag="sm", bufs=1, name="sm")[:]
            lsm = wk.tile([16, TB], F32, tag="lsm", bufs=1, name="lsm")[:]
            rc = wk.tile([16, TB], BF16, tag="rc", bufs=1, name="rc")[:]
            v.tensor_add(sm, ab[:, 0:TB], ab[:, TB:2 * TB])
            v.tensor_add(sm, sm, ab[:, 2 * TB:3 * TB])
            s.activation(lsm, sm, AF.Ln)
            s.activation(rc, lsm, AF.Exp, scale=-1.0)
            for j in range(3):
                abj = ab[:, j * TB:(j + 1) * TB]
                v.tensor_mul(abj, abj, rc)
            st["ab"] = ab

        def emit_v_xp(blk, st):
            """Attention-weighted V + residual -> xp (x16 units)."""
            ab = st["ab"]
            xp = wk.tile([128, KC * TB], BF16, tag="xp", bufs=1, name="xp")
            for mc in range(KC):
                pvs = []
                for j in range(3):
                    pvs.append(pp.tile([128, TB], F32, tag="ps", bufs=PSB,
                                       name=f"pv{j}"))
                for kp in range(KC // 2):
                    for j in range(3):
                        te.matmul(pvs[j][:],
                                  wvc[:, 2 * kp:2 * kp + 2,
                                      mc * 128:(mc + 1) * 128],
                                  st["mj3"][j][:, 2 * kp:2 * kp + 2, :],
                                  start=(kp == 0), stop=(kp == KC // 2 - 1),
                                  perf_mode=DR)
                acc = wk.tile([128, TB], BF16, tag="acc_s", bufs=1,
                              name="acc")
                for j in range(3):
                    vt = wk.tile([128, TB], BF16, tag="vt", bufs=2, name="vt")
                    v.tensor_copy(vt[:], pvs[j][:])
                    pa = pp.tile([128, TB], F32, tag="ps", bufs=PSB,
                                 name="pa")
                    te.matmul(pa[:], C["Eexp"][:, mc * 128:(mc + 1) * 128],
                              ab[:, j * TB:(j + 1) * TB],
                              start=True, stop=True)
                    if j == 0:
                        v.tensor_mul(acc[:], pa[:], vt[:])
                    else:
                        t2 = wk.tile([128, TB], BF16, tag="t2", bufs=2,
                                     name="t2")
                        v.tensor_mul(t2[:], pa[:], vt[:])
                        v.tensor_add(acc[:], acc[:], t2[:])
                # domb carries 16*(dom + bv); acc is 4096*sum_j a_j v_j
                v.scalar_tensor_tensor(
                    xp[:, mc * TB:(mc + 1) * TB], acc[:],
                    SA / SQK, st["domb"][:, mc * TB:(mc + 1) * TB],
                    ALU.mult, ALU.add)
            st["xp"] = xp

        def emit_ln1_finish(blk, st):
            pmub, prsb = _ln_finish(nc, pp, wk, C, st["pr1"], "a")
            xp = st["xp"]
            xf = wk.tile([128, KC * TB], BF16, tag="xf", bufs=1, name="xf")
            for kc in range(KC):
                xn = wk.tile([128, TB], BF16, tag="xn", bufs=2, name="xn")
                v.tensor_sub(xn[:], xp[:, kc * TB:(kc + 1) * TB], pmub[:])
                v.tensor_mul(xn[:], xn[:], prsb[:])
                s.activation(xf[:, kc * TB:(kc + 1) * TB], xn[:],
                             AF.Identity, scale=C["g1"][:, kc:kc + 1],
                             bias=C["be1"][:, kc:kc + 1])
            st["xf"] = xf

        def emit_ffn1(blk, st):
            """h = 16*relu(x@W1 + b1), W1 streamed; evictions split
            ACT/DVE to balance engine load."""
            xf = st["xf"]
            hb = wk.tile([128, MC1 * TB], BF16, tag="hb", bufs=1, name="hb")
            for mc in range(MC1):
                w1t = wk.tile([128, KC * 128], BF16, tag="w1s", bufs=4,
                              name="w1t")
                nc.sync.dma_start(
                    out=w1t[:],
                    in_=io["W1"][:, mc * KC * 128:(mc + 1) * KC * 128])
                ph = pp.tile([128, TB], F32, tag="ps", bufs=PSB, name="ph")
                for kc in range(KC):
                    te.matmul(ph[:], w1t[:, kc * 128:(kc + 1) * 128],
                              xf[:, kc * TB:(kc + 1) * TB],
                              start=(kc == 0), stop=(kc == KC - 1))
                dst = hb[:, mc * TB:(mc + 1) * TB]
                if mc % 2 == 0:
                    s.activation(dst, ph[:], AF.Relu,
                                 bias=C["b1"][:, mc:mc + 1])
                else:
                    v.tensor_scalar(dst, ph[:], C["b1"][:, mc:mc + 1], 0.0,
                                    ALU.add, ALU.max)
            st["hb"] = hb

        def emit_ffn2(blk, st):
            """x2 = 16*(x + h@W2 + b2), W2 streamed."""
            xf, hb = st["xf"], st["hb"]
            x2 = wk.tile([128, KC * TB], BF16, tag="x2", bufs=1, name="x2")
            for mp in range(KC // 2):
                tf = wk.tile([128, 2 * TB], BF16, tag="tf", bufs=1, name="tf")
                for half in range(2):
                    mc = 2 * mp + half
                    w2t = wk.tile([128, MC1 * 128], BF16, tag="w2s", bufs=2,
                                  name="w2t")
                    nc.sync.dma_start(
                        out=w2t[:],
                        in_=io["W2"][:, mc * MC1 * 128:(mc + 1) * MC1 * 128])
                    pf = pp.tile([128, TB], F32, tag="ps", bufs=PSB,
                                 name="pf")
                    for kc in range(MC1):
                        te.matmul(pf[:], w2t[:, kc * 128:(kc + 1) * 128],
                                  hb[:, kc * TB:(kc + 1) * TB],
                                  start=(kc == 0), stop=(kc == MC1 - 1))
                    s.activation(tf[:, half * TB:(half + 1) * TB], pf[:],
                                 AF.Identity, bias=C["b2"][:, mc:mc + 1])
                v.tensor_add(x2[:, 2 * mp * TB:(2 * mp + 2) * TB], tf[:],
                             xf[:, 2 * mp * TB:(2 * mp + 2) * TB])
            st["x2"] = x2

        def emit_ln2_finish_logits(blk, st):
            pmub2, prsb2 = _ln_finish(nc, pp, wk, C, st["pr2"], "b")
            x2 = st["x2"]
            pz = pp.tile([3, TB], F32, tag="ps", bufs=PSB, name="pz")
            for kc in range(KC):
                yn = wk.tile([128, TB], BF16, tag="xn", bufs=2, name="yn")
                v.tensor_sub(yn[:], x2[:, kc * TB:(kc + 1) * TB], pmub2[:])
                v.tensor_mul(yn[:], yn[:], prsb2[:])
                yb = wk.tile([128, TB], BF16, tag="yb", bufs=2, name="yb")
                s.activation(yb[:], yn[:], AF.Identity,
                             scale=C["g2"][:, kc:kc + 1],
                             bias=C["be2"][:, kc:kc + 1])
                te.matmul(pz[:], C["Wwt"][:, kc * 3:(kc + 1) * 3], yb[:],
                          start=(kc == 0), stop=(kc == KC - 1),
                          skip_group_check=True)
            zt = wk.tile([3, TB], F32, tag="zt", bufs=1, name="zt")
            s.activation(zt[:], pz[:], AF.Identity, bias=C["bw"][:])
            nc.sync.dma_start(out=io["zout"][:, blk * TB:(blk + 1) * TB],
                              in_=zt[:])

        # ---- software-pipelined emission: next-block attention fills the
        # LayerNorm stat necks of the current block ----
        sts = [None] * (nblk + 1)
        sts[0] = emit_inputs_q(0)
        emit_k_softmax(0, sts[0])
        for blk in range(nblk):
            st = sts[blk]
            emit_v_xp(blk, st)
            if blk > 0:
                emit_ln2_finish_logits(blk - 1, sts[blk - 1])
            if blk + 1 < nblk:
                sts[blk + 1] = emit_inputs_q(blk + 1)
            st["pr1"] = _ln_sums(nc, pp, wk, C, st["xp"], "a")
            if blk + 1 < nblk:
                emit_k_softmax(blk + 1, sts[blk + 1])
            emit_ln1_finish(blk, st)
            emit_ffn1(blk, st)
            emit_ffn2(blk, st)
            st["pr2"] = _ln_sums(nc, pp, wk, C, st["x2"], "b")
        emit_ln2_finish_logits(nblk - 1, sts[nblk - 1])


def build_program(tpc=TPC):
    nc = bacc.Bacc("TRN2", target_bir_lowering=False, debug=False)
    io = {}

    def din(name, shape, dtype):
        io[name] = nc.dram_tensor(name, shape, dtype, kind="ExternalInput").ap()

    nblk = tpc // TB
    din("qmv", [128, nblk * 2 * KC * TB], FP8)
    for j in range(3):
        din(f"m{j}", [128, nblk * KC * TB], FP8)
    din("domb", [128, nblk * KC * TB], BF16)
    din("Wqg", [128, 2 * KC * DIM], FP8)
    din("Wk", [128, KC * DIM], FP8)
    din("Wv", [128, KC * DIM], FP8)
    din("W1", [128, MC1 * KC * 128], BF16)
    din("W2", [128, KC * MC1 * 128], BF16)
    din("Ssel", [128, 128], BF16)
    din("Eexp", [16, 1024], BF16)
    din("onecb", [128, 1], BF16)
    din("onerb", [1, 128], BF16)
    for name, w in (("qbias", KC), ("bk", KC), ("bv", KC), ("b1", MC1),
                    ("b2", KC), ("g1", KC), ("be1", KC), ("g2", KC),
                    ("be2", KC)):
        din(name, [128, w], F32)
    din("Wwt", [128, 3 * KC], BF16)
    din("bw", [3, 1], F32)
    din("epsc", [1, 1], F32)
    io["zout"] = nc.dram_tensor("zout", [3, tpc], F32,
                                kind="ExternalOutput").ap()

    with tile.TileContext(nc) as tc:
        _emit(nc, tc, io, tpc)
    nc.compile()
    return nc


def _chunk_cols(vec, width):
    """[width*128] host vector -> [128, width] chunk-column layout."""
    return np.ascontiguousarray(
        np.asarray(vec, np.float32).reshape(width, 128).T)


def _chunk_major(w, scale):
    """[Din, N] weight -> [128, (Din/128)*N] fp8 chunk-major, scaled."""
    f8 = ml_dtypes.float8_e4m3
    din, n = w.shape
    kc = din // 128
    out = (np.asarray(w, np.float32) * scale).reshape(kc, 128, n)
    out = out.transpose(1, 0, 2).reshape(128, kc * n)
    return np.ascontiguousarray(np.clip(out, -224, 224)).astype(f8)


def _stream_layout(w, nin, nout):
    """[Din, Dout] -> [128, nout*nin*128] bf16 per-out-chunk streaming
    tiles: t[p, mc*nin*128 + kc*128 + n] = w[kc*128+p, mc*128+n]."""
    out = np.asarray(w, np.float32).reshape(nin, 128, nout, 128)
    out = out.transpose(1, 2, 0, 3).reshape(128, nout * nin * 128)
    return np.ascontiguousarray(out).astype(ml_dtypes.bfloat16)


def _act_blocks(x, tpc, scale, dtype):
    """[B, DIM] -> per-core list of [128, nblk*KC*TB] block-major tiles."""
    nblk = tpc // TB
    xs = (np.asarray(x, np.float32).T * scale)        # [DIM, B]
    out = []
    for c in range(xs.shape[1] // tpc):
        xc = xs[:, c * tpc:(c + 1) * tpc]             # [DIM, tpc]
        # a[p, blk*KC*TB + kc*TB + t] = xc[kc*128+p, blk*TB+t]
        a = xc.reshape(KC, 128, nblk, TB).transpose(1, 2, 0, 3)
        a = np.ascontiguousarray(a.reshape(128, nblk * KC * TB))
        if dtype == "f8":
            a = np.clip(a, -224, 224).astype(ml_dtypes.float8_e4m3)
        else:
            a = a.astype(ml_dtypes.bfloat16)
        out.append(a)
    return out


def prep_host_inputs(inputs, tpc=TPC, ncores=NCORES):
    f32 = np.float32
    bf = ml_dtypes.bfloat16
    rt = 1.0 / np.sqrt(HD)

    Wq = np.asarray(inputs["Wq"], f32) * rt
    Wg = np.asarray(inputs["Wg"], f32)
    Wgq = (Wg @ Wq) / 3.0
    qbias = (np.asarray(inputs["bg"], f32) @ Wq
             + np.asarray(inputs["bq"], f32) * rt)

    # head-selector S[p, mc*16+h] and expander E[h, mc*128+p]
    head_of = np.arange(DIM) // HD
    S = np.zeros((128, 128), f32)
    E = np.zeros((16, 1024), f32)
    for c in range(KC):
        for p in range(128):
            h = head_of[c * 128 + p]
            S[p, c * 16 + h] = 1.0
            E[h, c * 128 + p] = 1.0

    consts = {
        "Wqg": np.concatenate([_chunk_major(Wq, SWQ),
                               _chunk_major(Wgq, SWQ)], axis=1),
        "Wk": _chunk_major(np.asarray(inputs["Wk"], f32), SW),
        "Wv": _chunk_major(np.asarray(inputs["Wv"], f32), SW),
        "W1": _stream_layout(np.asarray(inputs["W1"], f32), KC, MC1),
        "W2": _stream_layout(np.asarray(inputs["W2"], f32), MC1, KC),
        "Ssel": S.astype(bf),
        "Eexp": E.astype(bf),
        "onecb": np.ones((128, 1), f32).astype(bf),
        "onerb": np.ones((1, 128), f32).astype(bf),
        "qbias": _chunk_cols(qbias / SQK, KC),
        "bk": _chunk_cols(np.asarray(inputs["bk"], f32) * SQK, KC),
        "bv": _chunk_cols(np.asarray(inputs["bv"], f32) * SA, KC),
        "b1": _chunk_cols(np.asarray(inputs["b1"], f32) * SA, MC1),
        "b2": _chunk_cols(np.asarray(inputs["b2"], f32) * SA, KC),
        "g1": _chunk_cols(np.asarray(inputs["g1"], f32) * SA, KC),
        "be1": _chunk_cols(np.asarray(inputs["beta1"], f32) * SA, KC),
        "g2": _chunk_cols(np.asarray(inputs["g2"], f32), KC),
        "be2": _chunk_cols(np.asarray(inputs["beta2"], f32), KC),
        "Wwt": np.ascontiguousarray(
            np.asarray(inputs["Ww"], f32).reshape(KC, 128, 3)
            .transpose(1, 0, 2).reshape(128, 3 * KC)).astype(bf),
        "bw": np.asarray(inputs["bw"], f32).reshape(3, 1),
        "epsc": np.full((1, 1), 256.0 * EPS, f32),
    }

    m0 = np.asarray(inputs["m0"], f32)
    m1 = np.asarray(inputs["m1"], f32)
    m2 = np.asarray(inputs["m2"], f32)
    dom = np.asarray(inputs["domain_rep"], f32)
    msum = m0 + m1 + m2

    m0b = _act_blocks(m0, tpc, SA, "f8")
    m1b = _act_blocks(m1, tpc, SA, "f8")
    m2b = _act_blocks(m2, tpc, SA, "f8")
    domf = _act_blocks(dom, tpc, SA, "f8")
    msf = _act_blocks(msum, tpc, SA, "f8")
    # domb carries the attention-path residual plus bv (sum_j attn_j == 1)
    domb = _act_blocks(dom + np.asarray(inputs["bv"], f32)[None, :],
                       tpc, SA, "bf")

    nblk = tpc // TB
    in_maps = []
    for c in range(ncores):
        m = dict(consts)
        # qmv: per block, dom's 8 chunks then msum's 8 chunks
        q = np.empty((128, nblk * 2 * KC * TB), ml_dtypes.float8_e4m3)
        for b_ in range(nblk):
            q[:, b_ * 2 * KC * TB:b_ * 2 * KC * TB + KC * TB] = \
                domf[c][:, b_ * KC * TB:(b_ + 1) * KC * TB]
            q[:, b_ * 2 * KC * TB + KC * TB:(b_ + 1) * 2 * KC * TB] = \
                msf[c][:, b_ * KC * TB:(b_ + 1) * KC * TB]
        m["qmv"] = q
        m["m0"] = m0b[c]
        m["m1"] = m1b[c]
        m["m2"] = m2b[c]
        m["domb"] = domb[c]
        in_maps.append(m)
    return in_maps


def kernel(**inputs):
    from concourse.bass_utils import run_bass_kernel_spmd
    nc = build_program()
    in_maps = prep_host_inputs(inputs)
    res = run_bass_kernel_spmd(nc, in_maps, list(range(NCORES)))
    zs = [res.results[c]["zout"] for c in range(NCORES)]
    z = np.concatenate([np.asarray(zc, np.float32).T for zc in zs], axis=0)
    z = z - z.max(axis=1, keepdims=True)
    e = np.exp(z)
    return np.ascontiguousarray(
        (e / e.sum(axis=1, keepdims=True)).astype(np.float32))


# revision 29
# speedup vs baseline: 1.3921x; 1.0615x over previous
"""Trainium2 Bass kernel for the fused 3-modality attention + FFN + softmax model.

v2: fp8e4 DoubleRow matmuls (2 k-tiles per PE pass) for all six big GEMMs,
single fused phase (all weights resident in SBUF, no DRAM roundtrip for x),
host-folded Wgq = Wg@Wq/(3*sqrt(HD)) to break the global_rep dependency,
activations carried in "x16 units" (LayerNorm is scale-invariant) so fp8
quantization scales fold into existing eviction scales, and rsqrt/reciprocal
computed as exp(-a*ln(x)) so the ACT engine stays on one table set
(natural_log_exp_and_others) with no table-swap stalls.

Layout: pure data parallel over 8 NeuronCores (batch sharded). Activations
feature-major on chip: [128 partitions, chunk*tokens] with DIM=1024 split
into KC=8 chunks of 128 partitions. LayerNorm reductions over features are
ones-vector matmuls on the PE; per-token scalars broadcast back with K=1
expand matmuls. Final 3-way softmax is done on the host (logits are DMA'd).
"""

import numpy as np
import ml_dtypes

import concourse.bacc as bacc
import concourse.bass as bass
import concourse.mybir as mybir
import concourse.tile as tile

B, DIM, H, FFN, HD = 16384, 1024, 16, 4096, 64
NCORES = 8
TPC = B // NCORES          # tokens per core
TB = 512                   # token block (matmul moving dim)
KC = DIM // 128            # 8 feature chunks
MC1 = FFN // 128           # 32 ffn chunks
EPS = 1e-5

# quantization scales (powers of two)
SA = 16.0                  # activation fp8 scale (x16 units)
SW = 256.0                 # weight fp8 scale for Wk/Wv/W1/W2
SWQ = 2048.0               # weight fp8 scale for Wq' and Wgq
SQK = 4096.0               # folded k-dequant carried inside qb

BF16 = mybir.dt.bfloat16
F32 = mybir.dt.float32
FP8 = mybir.dt.float8e4
AF = mybir.ActivationFunctionType
ALU = mybir.AluOpType
DR = mybir.MatmulPerfMode.DoubleRow


def _c3(t, nper):
    """[128, nchunk*nper] tile AP -> [128, nchunk, nper] view."""
    return t[:].rearrange("p (c n) -> p c n", n=nper)


def _ln_sums(nc, pp, wk, C, src_bf, tag):
    """Feature-axis sum (rows 0) and sum-of-squares (rows 32) matmuls into
    ONE psum bank via col-tiling. Returns the shared psum tile."""
    s, te = nc.scalar, nc.tensor
    pr = pp.tile([33, TB], F32, tag="ps", bufs=PSB, name=f"pr{tag}")
    for kc in range(KC):
        te.matmul(pr[0:1, :], C["onecb"][:], src_bf[:, kc * TB:(kc + 1) * TB],
                  start=(kc == 0), stop=(kc == KC - 1),
                  skip_group_check=True)
    for p2 in range(KC // 2):
        sq = wk.tile([128, 2 * TB], BF16, tag="sq", bufs=2, name="sq")
        s.activation(sq[:], src_bf[:, 2 * p2 * TB:(2 * p2 + 2) * TB],
                     AF.Square)
        for i in range(2):
            kc = 2 * p2 + i
            te.matmul(pr[32:33, :], C["onecb"][:],
                      sq[:, i * TB:(i + 1) * TB],
                      start=(kc == 0), stop=(kc == KC - 1),
                      skip_group_check=True)
    return pr


def _ln_stats(nc, pp, wk, C, pr, tag):
    """ACT/DVE-only stats chain from the sums bank; returns (mub, rsb)."""
    v, s = nc.vector, nc.scalar
    mub = wk.tile([1, TB], BF16, tag="ln_mub", bufs=2, name="mub")[:]
    ex2 = wk.tile([1, TB], F32, tag="ln_ex2", bufs=1, name="ex2")[:]
    mu2 = wk.tile([1, TB], F32, tag="ln_mu2", bufs=1, name="mu2")[:]
    var = wk.tile([1, TB], F32, tag="ln_var", bufs=1, name="var")[:]
    lnv = wk.tile([1, TB], F32, tag="ln_lnv", bufs=1, name="lnv")[:]
    rsb = wk.tile([1, TB], BF16, tag="ln_rsb", bufs=2, name="rsb")[:]
    s.activation(mub, pr[0:1, :], AF.Copy, scale=1.0 / DIM)
    s.activation(ex2, pr[32:33, :], AF.Copy, scale=1.0 / DIM)
    s.activation(mu2, mub, AF.Square)
    v.tensor_sub(var, ex2, mu2)
    # rs = exp(-0.5*ln(var + 256*eps)) : stays on the ln/exp ACT table set
    s.activation(lnv, var, AF.Ln, bias=C["epsc"][:])
    s.activation(rsb, lnv, AF.Exp, scale=-0.5)
    return mub, rsb


def _ln_bcast(nc, pp, wk, C, mub, rsb, tag):
    """Broadcast the per-token stats to 128 partitions (PE expands)."""
    s, te = nc.scalar, nc.tensor
    pmu = pp.tile([128, TB], F32, tag="ps", bufs=PSB, name=f"pmu{tag}")
    te.matmul(pmu[:], C["onerb"][:], mub, start=True, stop=True)
    prs = pp.tile([128, TB], F32, tag="ps", bufs=PSB, name=f"prs{tag}")
    te.matmul(prs[:], C["onerb"][:], rsb, start=True, stop=True)
    pmub = wk.tile([128, TB], BF16, tag="pmub", bufs=2, name="pmub")
    s.activation(pmub[:], pmu[:], AF.Copy)
    prsb = wk.tile([128, TB], BF16, tag="prsb", bufs=2, name="prsb")
    s.activation(prsb[:], prs[:], AF.Copy)
    return pmub, prsb


PSB = 8  # single rotating psum tag: all tiles are one bank each


def _emit(nc, tc, io, tpc):
    nblk = tpc // TB
    v, s, te = nc.vector, nc.scalar, nc.tensor

    with (
        tc.tile_pool(name="consts", bufs=1) as cp,
        tc.tile_pool(name="weights", bufs=1) as wp,
        tc.tile_pool(name="psum", bufs=1, space="PSUM") as pp,
        tc.tile_pool(name="work", bufs=1) as wk,
    ):
        # pin the ACT table to natural_log_exp_and_others (covers Copy/
        # Identity/Relu/Square/Exp/Ln) so the greedy per-func chooser never
        # flip-flops tables mid-kernel.
        nc.scalar.add_instruction(mybir.InstLoadActFuncSet(
            name=nc.get_next_instruction_name(), ins=[], outs=[],
            act_func_set_id=6))

        # ---- small constants ----
        C = {}
        for name, shape, dtype in (
            ("Ssel", [128, 128], BF16), ("Eexp", [16, 1024], BF16),
            ("onecb", [128, 1], BF16), ("onerb", [1, 128], BF16),
            ("qbias", [128, KC], F32), ("bk", [128, KC], F32),
            ("b1", [128, MC1], F32), ("b2", [128, KC], F32),
            ("g1", [128, KC], F32), ("be1", [128, KC], F32),
            ("g2", [128, KC], F32), ("be2", [128, KC], F32),
            ("Wwt", [128, 3 * KC], BF16), ("bw", [3, 1], F32),
            ("epsc", [1, 1], F32),
        ):
            t = cp.tile(shape, dtype, name=f"c_{name}")
            nc.sync.dma_start(out=t[:], in_=io[name])
            C[name] = t

        # ---- resident fp8 weights (chunk-major: w[p, kc*N + n]) ----
        W = {}
        for name, width in (("Wqg", 2 * KC * DIM), ("Wk", KC * DIM),
                            ("Wv", KC * DIM)):
            t = wp.tile([128, width], FP8, name=f"w_{name}")
            nc.sync.dma_start(out=t[:], in_=io[name])
            W[name] = t
        wqg = _c3(W["Wqg"], DIM)
        wkc = _c3(W["Wk"], DIM)
        wvc = _c3(W["Wv"], DIM)

        def emit_inputs_q(blk):
            """DMA block inputs; q = dom@Wq' + msum@Wgq (2048-deep)."""
            t0 = blk * KC * TB
            st = {}
            qmv = wk.tile([128, 2 * KC * TB], FP8, tag="qmv", bufs=1,
                          name="qmv")
            nc.sync.dma_start(out=qmv[:],
                              in_=io["qmv"][:, 2 * t0:2 * t0 + 2 * KC * TB])
            mj = []
            for j in range(3):
                t = wk.tile([128, KC * TB], FP8, tag=f"m{j}", bufs=2,
                            name=f"m{j}")
                nc.sync.dma_start(out=t[:],
                                  in_=io[f"m{j}"][:, t0:t0 + KC * TB])
                mj.append(t)
            domb = wk.tile([128, KC * TB], BF16, tag="domb", bufs=1,
                           name="domb")
            nc.sync.dma_start(out=domb[:],
                              in_=io["domb"][:, t0:t0 + KC * TB])
            st["qmv3"] = _c3(qmv, TB)
            st["mj3"] = [_c3(t, TB) for t in mj]
            st["domb"] = domb
            qb = wk.tile([128, KC * TB], BF16, tag="qb", bufs=1, name="qb")
            for mc in range(KC):
                pq = pp.tile([128, TB], F32, tag="ps", bufs=PSB, name="pq")
                for kp in range(KC):
                    te.matmul(pq[:],
                              wqg[:, 2 * kp:2 * kp + 2,
                                  mc * 128:(mc + 1) * 128],
                              st["qmv3"][:, 2 * kp:2 * kp + 2, :],
                              start=(kp == 0), stop=(kp == KC - 1),
                              perf_mode=DR)
                s.activation(qb[:, mc * TB:(mc + 1) * TB], pq[:],
                             AF.Identity, scale=1.0 / (SA * SWQ * SQK),
                             bias=C["qbias"][:, mc:mc + 1])
            st["qb"] = qb
            return st

        def emit_k_softmax(blk, st):
            """Scores into one col-tiled psum bank; selects lag one chunk so
            the PE never waits on the tm STT; then the 3-way softmax."""
            qb = st["qb"]
            sca = pp.tile([96, TB], F32, tag="ps", bufs=PSB, name="sca")
            pend = None
            for mc in range(KC):
                pks = []
                for j in range(3):
                    pks.append(pp.tile([128, TB], F32, tag="ps", bufs=PSB,
                                       name=f"pk{j}"))
                for kp in range(KC // 2):
                    for j in range(3):
                        te.matmul(pks[j][:],
                                  wkc[:, 2 * kp:2 * kp + 2,
                                      mc * 128:(mc + 1) * 128],
                                  st["mj3"][j][:, 2 * kp:2 * kp + 2, :],
                                  start=(kp == 0), stop=(kp == KC // 2 - 1),
                                  perf_mode=DR)
                tms = []
                for j in range(3):
                    tm = wk.tile([128, TB], BF16, tag="tm", bufs=6, name="tm")
                    v.scalar_tensor_tensor(
                        tm[:], pks[j][:], C["bk"][:, mc:mc + 1],
                        qb[:, mc * TB:(mc + 1) * TB], ALU.add, ALU.mult)
                    tms.append(tm)

                def sel(pmc, ptms):
                    for j in range(3):
                        te.matmul(sca[32 * j:32 * j + 16, :],
                                  C["Ssel"][:, pmc * 16:(pmc + 1) * 16],
                                  ptms[j][:],
                                  start=(pmc == 0),
                                  stop=(pmc == KC - 1),
                                  skip_group_check=True)
                if pend is not None:
                    sel(*pend)
                pend = (mc, tms)
            sel(*pend)

            ab = wk.tile([16, 3 * TB], BF16, tag="ab", bufs=1, name="ab")
            for j in range(3):
                s.activation(ab[:, j * TB:(j + 1) * TB],
                             sca[32 * j:32 * j + 16, :], AF.Exp)
            sm = wk.tile([16, TB], BF16, tag="sm", bufs=1, name="sm")[:]
            lsm = wk.tile([16, TB], F32, tag="lsm", bufs=1, name="lsm")[:]
            rc = wk.tile([16, TB], BF16, tag="rc", bufs=1, name="rc")[:]
            v.tensor_add(sm, ab[:, 0:TB], ab[:, TB:2 * TB])
            v.tensor_add(sm, sm, ab[:, 2 * TB:3 * TB])
            s.activation(lsm, sm, AF.Ln)
            s.activation(rc, lsm, AF.Exp, scale=-1.0)
            for j in range(3):
                abj = ab[:, j * TB:(j + 1) * TB]
                v.tensor_mul(abj, abj, rc)
            st["ab"] = ab

        def emit_v_xp(blk, st):
            """Attention-weighted V + residual -> xp (x16 units)."""
            ab = st["ab"]
            xp = wk.tile([128, KC * TB], BF16, tag="xp", bufs=1, name="xp")
            for mc in range(KC):
                pvs = []
                for j in range(3):
                    pvs.append(pp.tile([128, TB], F32, tag="ps", bufs=PSB,
                                       name=f"pv{j}"))
                for kp in range(KC // 2):
                    for j in range(3):
                        te.matmul(pvs[j][:],
                                  wvc[:, 2 * kp:2 * kp + 2,
                                      mc * 128:(mc + 1) * 128],
                                  st["mj3"][j][:, 2 * kp:2 * kp + 2, :],
                                  start=(kp == 0), stop=(kp == KC // 2 - 1),
                                  perf_mode=DR)
                acc = wk.tile([128, TB], BF16, tag="acc_s", bufs=1,
                              name="acc")
                for j in range(3):
                    vt = wk.tile([128, TB], BF16, tag="vt", bufs=2, name="vt")
                    s.activation(vt[:], pvs[j][:], AF.Copy)
                    pa = pp.tile([128, TB], F32, tag="ps", bufs=PSB,
                                 name="pa")
                    te.matmul(pa[:], C["Eexp"][:, mc * 128:(mc + 1) * 128],
                              ab[:, j * TB:(j + 1) * TB],
                              start=True, stop=True)
                    if j == 0:
                        v.tensor_mul(acc[:], pa[:], vt[:])
                    else:
                        t2 = wk.tile([128, TB], BF16, tag="t2", bufs=2,
                                     name="t2")
                        v.tensor_mul(t2[:], pa[:], vt[:])
                        v.tensor_add(acc[:], acc[:], t2[:])
                # domb carries 16*(dom + bv); acc is 4096*sum_j a_j v_j
                v.scalar_tensor_tensor(
                    xp[:, mc * TB:(mc + 1) * TB], acc[:],
                    SA / SQK, st["domb"][:, mc * TB:(mc + 1) * TB],
                    ALU.mult, ALU.add)
            st["xp"] = xp

        def emit_ln1_finish(blk, st):
            pmub, prsb = _ln_bcast(nc, pp, wk, C, *st["st1"], "a")
            xp = st["xp"]
            xf = wk.tile([128, KC * TB], BF16, tag="xf", bufs=1, name="xf")
            for kc in range(KC):
                xn = wk.tile([128, TB], BF16, tag="xn", bufs=2, name="xn")
                v.tensor_sub(xn[:], xp[:, kc * TB:(kc + 1) * TB], pmub[:])
                v.tensor_mul(xn[:], xn[:], prsb[:])
                s.activation(xf[:, kc * TB:(kc + 1) * TB], xn[:],
                             AF.Identity, scale=C["g1"][:, kc:kc + 1],
                             bias=C["be1"][:, kc:kc + 1])
            st["xf"] = xf

        def emit_ffn1(blk, st):
            """h = 16*relu(x@W1 + b1), W1 streamed; evictions split
            ACT/DVE to balance engine load."""
            xf = st["xf"]
            hb = wk.tile([128, MC1 * TB], BF16, tag="hb", bufs=1, name="hb")
            for mc in range(MC1):
                w1t = wk.tile([128, KC * 128], BF16, tag="w1s", bufs=3,
                              name="w1t")
                nc.sync.dma_start(
                    out=w1t[:],
                    in_=io["W1"][:, mc * KC * 128:(mc + 1) * KC * 128])
                ph = pp.tile([128, TB], F32, tag="ps", bufs=PSB, name="ph")
                for kc in range(KC):
                    te.matmul(ph[:], w1t[:, kc * 128:(kc + 1) * 128],
                              xf[:, kc * TB:(kc + 1) * TB],
                              start=(kc == 0), stop=(kc == KC - 1))
                dst = hb[:, mc * TB:(mc + 1) * TB]
                if mc % 2 == 0:
                    s.activation(dst, ph[:], AF.Relu,
                                 bias=C["b1"][:, mc:mc + 1])
                else:
                    v.tensor_scalar(dst, ph[:], C["b1"][:, mc:mc + 1], 0.0,
                                    ALU.add, ALU.max)
            st["hb"] = hb

        def emit_ffn2(blk, st):
            """x2 = 16*(x + h@W2 + b2), W2 streamed."""
            xf, hb = st["xf"], st["hb"]
            x2 = wk.tile([128, KC * TB], BF16, tag="x2", bufs=1, name="x2")
            for mp in range(KC // 2):
                tf = wk.tile([128, 2 * TB], BF16, tag="tf", bufs=1, name="tf")
                for half in range(2):
                    mc = 2 * mp + half
                    w2t = wk.tile([128, MC1 * 128], BF16, tag="w2s", bufs=2,
                                  name="w2t")
                    nc.sync.dma_start(
                        out=w2t[:],
                        in_=io["W2"][:, mc * MC1 * 128:(mc + 1) * MC1 * 128])
                    pf = pp.tile([128, TB], F32, tag="ps", bufs=PSB,
                                 name="pf")
                    for kc in range(MC1):
                        te.matmul(pf[:], w2t[:, kc * 128:(kc + 1) * 128],
                                  hb[:, kc * TB:(kc + 1) * TB],
                                  start=(kc == 0), stop=(kc == MC1 - 1))
                    s.activation(tf[:, half * TB:(half + 1) * TB], pf[:],
                                 AF.Identity, bias=C["b2"][:, mc:mc + 1])
                v.tensor_add(x2[:, 2 * mp * TB:(2 * mp + 2) * TB], tf[:],
                             xf[:, 2 * mp * TB:(2 * mp + 2) * TB])
            st["x2"] = x2

        def emit_ln2_finish_logits(blk, st):
            pmub2, prsb2 = _ln_bcast(nc, pp, wk, C, *st["st2"], "b")
            x2 = st["x2"]
            pz = pp.tile([3, TB], F32, tag="ps", bufs=PSB, name="pz")
            for kc in range(KC):
                yn = wk.tile([128, TB], BF16, tag="xn", bufs=2, name="yn")
                v.tensor_sub(yn[:], x2[:, kc * TB:(kc + 1) * TB], pmub2[:])
                v.tensor_mul(yn[:], yn[:], prsb2[:])
                yb = wk.tile([128, TB], BF16, tag="yb", bufs=2, name="yb")
                s.activation(yb[:], yn[:], AF.Identity,
                             scale=C["g2"][:, kc:kc + 1],
                             bias=C["be2"][:, kc:kc + 1])
                te.matmul(pz[:], C["Wwt"][:, kc * 3:(kc + 1) * 3], yb[:],
                          start=(kc == 0), stop=(kc == KC - 1),
                          skip_group_check=True)
            zt = wk.tile([3, TB], F32, tag="zt", bufs=1, name="zt")
            s.activation(zt[:], pz[:], AF.Identity, bias=C["bw"][:])
            nc.sync.dma_start(out=io["zout"][:, blk * TB:(blk + 1) * TB],
                              in_=zt[:])

        # ---- software-pipelined emission: next-block attention fills the
        # LayerNorm stat necks of the current block ----
        sts = [None] * (nblk + 1)
        sts[0] = emit_inputs_q(0)
        emit_k_softmax(0, sts[0])
        for blk in range(nblk):
            st = sts[blk]
            if blk > 0:
                # LN2(blk-1) stats on ACT/DVE before the v-phase floods them
                sts[blk - 1]["st2"] = _ln_stats(
                    nc, pp, wk, C, sts[blk - 1]["pr2"], "b")
            emit_v_xp(blk, st)
            if blk > 0:
                emit_ln2_finish_logits(blk - 1, sts[blk - 1])
            if blk + 1 < nblk:
                sts[blk + 1] = emit_inputs_q(blk + 1)
            st["pr1"] = _ln_sums(nc, pp, wk, C, st["xp"], "a")
            st["st1"] = _ln_stats(nc, pp, wk, C, st["pr1"], "a")
            if blk + 1 < nblk:
                emit_k_softmax(blk + 1, sts[blk + 1])
            emit_ln1_finish(blk, st)
            emit_ffn1(blk, st)
            emit_ffn2(blk, st)
            st["pr2"] = _ln_sums(nc, pp, wk, C, st["x2"], "b")
        sts[nblk - 1]["st2"] = _ln_stats(nc, pp, wk, C,
                                         sts[nblk - 1]["pr2"], "b")
        emit_ln2_finish_logits(nblk - 1, sts[nblk - 1])


def build_program(tpc=TPC):
    nc = bacc.Bacc("TRN2", target_bir_lowering=False, debug=False)
    io = {}

    def din(name, shape, dtype):
        io[name] = nc.dram_tensor(name, shape, dtype, kind="ExternalInput").ap()

    nblk = tpc // TB
    din("qmv", [128, nblk * 2 * KC * TB], FP8)
    for j in range(3):
        din(f"m{j}", [128, nblk * KC * TB], FP8)
    din("domb", [128, nblk * KC * TB], BF16)
    din("Wqg", [128, 2 * KC * DIM], FP8)
    din("Wk", [128, KC * DIM], FP8)
    din("Wv", [128, KC * DIM], FP8)
    din("W1", [128, MC1 * KC * 128], BF16)
    din("W2", [128, KC * MC1 * 128], BF16)
    din("Ssel", [128, 128], BF16)
    din("Eexp", [16, 1024], BF16)
    din("onecb", [128, 1], BF16)
    din("onerb", [1, 128], BF16)
    for name, w in (("qbias", KC), ("bk", KC), ("bv", KC), ("b1", MC1),
                    ("b2", KC), ("g1", KC), ("be1", KC), ("g2", KC),
                    ("be2", KC)):
        din(name, [128, w], F32)
    din("Wwt", [128, 3 * KC], BF16)
    din("bw", [3, 1], F32)
    din("epsc", [1, 1], F32)
    io["zout"] = nc.dram_tensor("zout", [3, tpc], F32,
                                kind="ExternalOutput").ap()

    with tile.TileContext(nc) as tc:
        _emit(nc, tc, io, tpc)
    nc.compile()
    return nc


def _chunk_cols(vec, width):
    """[width*128] host vector -> [128, width] chunk-column layout."""
    return np.ascontiguousarray(
        np.asarray(vec, np.float32).reshape(width, 128).T)


def _chunk_major(w, scale):
    """[Din, N] weight -> [128, (Din/128)*N] fp8 chunk-major, scaled."""
    f8 = ml_dtypes.float8_e4m3
    din, n = w.shape
    kc = din // 128
    out = (np.asarray(w, np.float32) * scale).reshape(kc, 128, n)
    out = out.transpose(1, 0, 2).reshape(128, kc * n)
    return np.ascontiguousarray(np.clip(out, -224, 224)).astype(f8)


def _stream_layout(w, nin, nout):
    """[Din, Dout] -> [128, nout*nin*128] bf16 per-out-chunk streaming
    tiles: t[p, mc*nin*128 + kc*128 + n] = w[kc*128+p, mc*128+n]."""
    out = np.asarray(w, np.float32).reshape(nin, 128, nout, 128)
    out = out.transpose(1, 2, 0, 3).reshape(128, nout * nin * 128)
    return np.ascontiguousarray(out).astype(ml_dtypes.bfloat16)


def _act_blocks(x, tpc, scale, dtype):
    """[B, DIM] -> per-core list of [128, nblk*KC*TB] block-major tiles."""
    nblk = tpc // TB
    xs = (np.asarray(x, np.float32).T * scale)        # [DIM, B]
    out = []
    for c in range(xs.shape[1] // tpc):
        xc = xs[:, c * tpc:(c + 1) * tpc]             # [DIM, tpc]
        # a[p, blk*KC*TB + kc*TB + t] = xc[kc*128+p, blk*TB+t]
        a = xc.reshape(KC, 128, nblk, TB).transpose(1, 2, 0, 3)
        a = np.ascontiguousarray(a.reshape(128, nblk * KC * TB))
        if dtype == "f8":
            a = np.clip(a, -224, 224).astype(ml_dtypes.float8_e4m3)
        else:
            a = a.astype(ml_dtypes.bfloat16)
        out.append(a)
    return out


def prep_host_inputs(inputs, tpc=TPC, ncores=NCORES):
    f32 = np.float32
    bf = ml_dtypes.bfloat16
    rt = 1.0 / np.sqrt(HD)

    Wq = np.asarray(inputs["Wq"], f32) * rt
    Wg = np.asarray(inputs["Wg"], f32)
    Wgq = (Wg @ Wq) / 3.0
    qbias = (np.asarray(inputs["bg"], f32) @ Wq
             + np.asarray(inputs["bq"], f32) * rt)

    # head-selector S[p, mc*16+h] and expander E[h, mc*128+p]
    head_of = np.arange(DIM) // HD
    S = np.zeros((128, 128), f32)
    E = np.zeros((16, 1024), f32)
    for c in range(KC):
        for p in range(128):
            h = head_of[c * 128 + p]
            S[p, c * 16 + h] = 1.0
            E[h, c * 128 + p] = 1.0

    consts = {
        "Wqg": np.concatenate([_chunk_major(Wq, SWQ),
                               _chunk_major(Wgq, SWQ)], axis=1),
        "Wk": _chunk_major(np.asarray(inputs["Wk"], f32), SW),
        "Wv": _chunk_major(np.asarray(inputs["Wv"], f32), SW),
        "W1": _stream_layout(np.asarray(inputs["W1"], f32), KC, MC1),
        "W2": _stream_layout(np.asarray(inputs["W2"], f32), MC1, KC),
        "Ssel": S.astype(bf),
        "Eexp": E.astype(bf),
        "onecb": np.ones((128, 1), f32).astype(bf),
        "onerb": np.ones((1, 128), f32).astype(bf),
        "qbias": _chunk_cols(qbias / SQK, KC),
        "bk": _chunk_cols(np.asarray(inputs["bk"], f32) * SQK, KC),
        "bv": _chunk_cols(np.asarray(inputs["bv"], f32) * SA, KC),
        "b1": _chunk_cols(np.asarray(inputs["b1"], f32) * SA, MC1),
        "b2": _chunk_cols(np.asarray(inputs["b2"], f32) * SA, KC),
        "g1": _chunk_cols(np.asarray(inputs["g1"], f32) * SA, KC),
        "be1": _chunk_cols(np.asarray(inputs["beta1"], f32) * SA, KC),
        "g2": _chunk_cols(np.asarray(inputs["g2"], f32), KC),
        "be2": _chunk_cols(np.asarray(inputs["beta2"], f32), KC),
        "Wwt": np.ascontiguousarray(
            np.asarray(inputs["Ww"], f32).reshape(KC, 128, 3)
            .transpose(1, 0, 2).reshape(128, 3 * KC)).astype(bf),
        "bw": np.asarray(inputs["bw"], f32).reshape(3, 1),
        "epsc": np.full((1, 1), 256.0 * EPS, f32),
    }

    m0 = np.asarray(inputs["m0"], f32)
    m1 = np.asarray(inputs["m1"], f32)
    m2 = np.asarray(inputs["m2"], f32)
    dom = np.asarray(inputs["domain_rep"], f32)
    msum = m0 + m1 + m2

    m0b = _act_blocks(m0, tpc, SA, "f8")
    m1b = _act_blocks(m1, tpc, SA, "f8")
    m2b = _act_blocks(m2, tpc, SA, "f8")
    domf = _act_blocks(dom, tpc, SA, "f8")
    msf = _act_blocks(msum, tpc, SA, "f8")
    # domb carries the attention-path residual plus bv (sum_j attn_j == 1)
    domb = _act_blocks(dom + np.asarray(inputs["bv"], f32)[None, :],
                       tpc, SA, "bf")

    nblk = tpc // TB
    in_maps = []
    for c in range(ncores):
        m = dict(consts)
        # qmv: per block, dom's 8 chunks then msum's 8 chunks
        q = np.empty((128, nblk * 2 * KC * TB), ml_dtypes.float8_e4m3)
        for b_ in range(nblk):
            q[:, b_ * 2 * KC * TB:b_ * 2 * KC * TB + KC * TB] = \
                domf[c][:, b_ * KC * TB:(b_ + 1) * KC * TB]
            q[:, b_ * 2 * KC * TB + KC * TB:(b_ + 1) * 2 * KC * TB] = \
                msf[c][:, b_ * KC * TB:(b_ + 1) * KC * TB]
        m["qmv"] = q
        m["m0"] = m0b[c]
        m["m1"] = m1b[c]
        m["m2"] = m2b[c]
        m["domb"] = domb[c]
        in_maps.append(m)
    return in_maps


def kernel(**inputs):
    from concourse.bass_utils import run_bass_kernel_spmd
    nc = build_program()
    in_maps = prep_host_inputs(inputs)
    res = run_bass_kernel_spmd(nc, in_maps, list(range(NCORES)))
    zs = [res.results[c]["zout"] for c in range(NCORES)]
    z = np.concatenate([np.asarray(zc, np.float32).T for zc in zs], axis=0)
    z = z - z.max(axis=1, keepdims=True)
    e = np.exp(z)
    return np.ascontiguousarray(
        (e / e.sum(axis=1, keepdims=True)).astype(np.float32))
